# revision 15
# baseline (speedup 1.0000x reference)
"""Multi-head self-attention (B=8, S=2048, H=256, NH=8, HD=32) on 8 TRN2 cores.

v3: data-parallel over batch (1 batch element/core). Changes over v2
(325us): the PE streams at full rate when busy (~0.42ns/col), so v3
attacks the ~70us of PE idle plus ~25us of excess streamed columns:
  - 8 q-blocks of 256 with DOUBLE-BUFFERED ctx PSUM (2x2 banks + 4-bank
    score ring): next block's ctx accumulation never waits on the
    previous block's eviction chain.
  - flat global group stream; ctx trails scores by CTX_DELAY groups;
    block-boundary work (evict -> rowsum recip roundtrip -> normalize ->
    out-proj -> store) is emitted at staggered slots so no engine stalls
    in program order.
  - evict/recip/broadcast in bf16 (halves boundary DMA, 2x_1p DVE
    normalize); q/k bias folded into phase-1 evictions as per-partition
    vectors (drops 16 rank-1 matmuls); out-projection as single 128-wide
    stationary tiles (halves its streamed columns); exp in [128,1024]
    units to amortize per-op overhead.
"""
import numpy as np
import ml_dtypes

import bass_rust
import concourse.bass as bass
import concourse.mybir as mybir
import concourse.tile as tile
from concourse.bass_utils import run_bass_kernel_spmd

BF16 = mybir.dt.bfloat16
F32 = mybir.dt.float32
I16 = mybir.dt.int16
NPBF16 = ml_dtypes.bfloat16

B, S, H = 8, 2048, 256
NH, HD = 8, 32
SCALE = 1.0 / float(np.sqrt(HD))
N_CORES = 8

LOG2E = 1.4426950408889634
# DVE bit-hack exp constants (c centers the mantissa-interp sawtooth; the
# +0.5 assumes truncation on float->int convert)
HACK_C = 5.5
HACK_ROUND = True
A_HACK = float(LOG2E * 128.0 * SCALE)
B_HACK = float(127.0 * 128.0 - HACK_C + (0.0 if HACK_ROUND else 0.5))

NQB = 8          # q-blocks
QBS = 256        # q-block size
NG = NQB * 32    # global group count (32 (g,kt) groups per block)
CTX_DELAY = 7    # groups between a group's scores and its ctx matmuls
EV_OFF = 1       # boundary stagger (slots after a block's last ctx batch)
NORM_OFF = 6
PROJ_OFF = 8
ACT_SHARE = 9    # of every 16 exp units, this many go to ScalarE

TRACE_OPTS = {}
LAST_RESULT = None
DEBUG_DUMPS = False
# HW bisect flags (safe = v2-style)
OUTPROJ_SPLIT = False   # True: out-proj as 2x64-wide col tiles (v2 style)
PLAIN_EVICT = False     # True: phase-1 evict without bias vectors
EXP_512 = False         # True: exp in [128,512] single-bank units
NO_RECIP = False        # True: skip recip chain (numerics wrong; crash bisect)
SKIP_P3 = False         # True: only phases 1/2 + dummy out (crash bisect)
MAX_GI = None           # truncate phase-3 slot loop (crash bisect)


def _legalize_sync_waits(nc):
    """Split multi-wait sync_infos onto NoOp carriers (walrus allows 1/inst)."""
    n = 0
    for f in nc.m.functions:
        for bb in f.blocks:
            insts = bb.instructions
            i = 0
            while i < len(insts):
                inst = insts[i]
                si = inst.sync_info
                if si is not None and len(si.on_wait) > 1:
                    waits = list(si.on_wait)
                    carriers = []
                    for w in waits[:-1]:
                        carriers.append(
                            mybir.InstNoOp(
                                name=f"{inst.name}-w{n}",
                                sync_info=mybir.SyncInfo(on_wait=[w], on_update=[]),
                                bass_nofuse=True,
                                engine=inst.engine,
                            )
                        )
                        n += 1
                    inst.sync_info = bass_rust.SyncInfo(
                        on_wait=waits[-1:], on_update=list(si.on_update)
                    )
                    insts[i:i] = carriers
                    i += len(carriers)
                i += 1
    return n


def _build_nc(legalize=True):
    nc = bass.Bass()
    xt = nc.dram_tensor("xt", [128, 2 * S], BF16, kind="ExternalInput")
    wqk = nc.dram_tensor("wqk", [128, 2 * 512], BF16, kind="ExternalInput")
    bqk = nc.dram_tensor("bqk", [128, 4], F32, kind="ExternalInput")
    bv = nc.dram_tensor("bv", [1, 264], BF16, kind="ExternalInput")
    wv = nc.dram_tensor("wv", [128, 2 * 264], BF16, kind="ExternalInput")
    wo = nc.dram_tensor("wo", [128, 4 * 256], BF16, kind="ExternalInput")
    ones = nc.dram_tensor("ones", [1, 512], BF16, kind="ExternalInput")
    out = nc.dram_tensor("out", [S, H], F32, kind="ExternalOutput")
    # rowsum gather / reciprocal round-trip scratch ([2,1024] <-> [128,16])
    rscr = nc.dram_tensor("rscr", [2, 1024], BF16)
    rscr2 = nc.dram_tensor("rscr2", [2, 1024], BF16)
    if DEBUG_DUMPS:
        dbg = {
            "dbg_qT": nc.dram_tensor("dbg_qT", [128, 2 * S], BF16, kind="ExternalOutput"),
            "dbg_kT": nc.dram_tensor("dbg_kT", [128, 2 * S], BF16, kind="ExternalOutput"),
            "dbg_v": nc.dram_tensor("dbg_v", [128, 16 * 264], BF16, kind="ExternalOutput"),
            "dbg_eT": nc.dram_tensor("dbg_eT", [128, 1024], BF16, kind="ExternalOutput"),
            "dbg_stg": nc.dram_tensor("dbg_stg", [128, 1024], BF16, kind="ExternalOutput"),
            "dbg_rcb": nc.dram_tensor("dbg_rcb", [128, 1024], BF16, kind="ExternalOutput"),
            "dbg_ctxT": nc.dram_tensor("dbg_ctxT", [128, 1024], BF16, kind="ExternalOutput"),
        }

    EXP = mybir.ActivationFunctionType.Exp
    IDENT = mybir.ActivationFunctionType.Identity

    with tile.TileContext(nc) as tc:
        with (
            tc.tile_pool(name="const", bufs=1) as const,
            tc.tile_pool(name="etp", bufs=8) as etp,
            tc.tile_pool(name="ctp", bufs=2) as ctp,
            tc.tile_pool(name="stp", bufs=2) as stp,
            tc.tile_pool(name="osb", bufs=2) as osb,
        ):
            # ---- input DMAs, in first-use order ----
            wqk_sb = const.tile([128, 2 * 512], BF16, tag="wqk")
            nc.sync.dma_start(out=wqk_sb, in_=wqk[:, :])
            xt_sb = const.tile([128, 2 * S], BF16, tag="xt")
            for ch in (0, 2, 1, 3):  # phase 1 consumes ks-paired halves
                nc.sync.dma_start(
                    out=xt_sb[:, ch * 1024: ch * 1024 + 1024],
                    in_=xt[:, ch * 1024: ch * 1024 + 1024])
            bqk_sb = const.tile([128, 4], F32, tag="bqk")
            nc.sync.dma_start(out=bqk_sb, in_=bqk[:, :])
            wv_sb = const.tile([128, 2 * 264], BF16, tag="wv")
            nc.sync.dma_start(out=wv_sb, in_=wv[:, :])
            bv_sb = const.tile([1, 264], BF16, tag="bv")
            nc.sync.dma_start(out=bv_sb, in_=bv[:, :])
            ones_sb = const.tile([1, 512], BF16, tag="ones")
            nc.sync.dma_start(out=ones_sb, in_=ones[:, :])
            wo_sb = const.tile([128, 4 * 256], BF16, tag="wo")
            nc.sync.dma_start(out=wo_sb, in_=wo[:, :])

            qT_sb = const.tile([128, 2 * S], BF16, tag="qT")
            kT_sb = const.tile([128, 2 * S], BF16, tag="kT")
            v_sb = const.tile([128, 16 * 264], BF16, tag="v")

            # persistent PSUM: double-buffered ctx accumulators (2x2 banks)
            with (
                tc.tile_pool(name="cxp", bufs=1, space="PSUM") as cxp,
            ):
                ctx_bufs = [
                    cxp.tile([128, 1024], F32, tag="ctxA", name="ctxA"),
                    cxp.tile([128, 1024], F32, tag="ctxB", name="ctxB"),
                ]

                # ---- warmup (p-state ramp) while input DMAs land ----
                warm_sb = const.tile([128, 512], BF16, tag="warm")
                nc.vector.memset(warm_sb, 0.0)
                for r in range(10):
                    nc.tensor.matmul(
                        out=ctx_bufs[0][:, 0:512], lhsT=warm_sb[:, 0:128],
                        rhs=warm_sb[:, :], start=True, stop=True,
                    )
                # zero the never-matmul-written ctx rows so full-height
                # normalize reads finite values
                for cb in ctx_bufs:
                    nc.vector.memset(cb[32:64, :], 0.0)
                    nc.vector.memset(cb[96:128, :], 0.0)

                # ---- phase 1: qT/kT [feature, s]; bias folded into the
                #      evictions as per-partition vectors ----
                p12 = tc.tile_pool(name="p12", bufs=4, space="PSUM")
                scp12 = p12.__enter__()
                for nb in range(4):  # s blocks of 512
                    for t in range(4):  # feature tiles: q0,q1,k0,k1
                        ps = scp12.tile([128, 512], F32, tag="sc",
                                        name=f"p1_{t}_{nb}")
                        for ks in range(2):
                            nc.tensor.matmul(
                                out=ps,
                                lhsT=wqk_sb[:, ks * 512 + t * 128: ks * 512 + t * 128 + 128],
                                rhs=xt_sb[:, ks * S + nb * 512: ks * S + nb * 512 + 512],
                                start=(ks == 0), stop=(ks == 1),
                            )
                        dst = (qT_sb if t < 2 else kT_sb)[
                            :, (t % 2) * S + nb * 512: (t % 2) * S + nb * 512 + 512
                        ]
                        if PLAIN_EVICT:
                            if t % 2 == 0:
                                nc.scalar.copy(out=dst, in_=ps)
                            else:
                                nc.vector.tensor_copy(out=dst, in_=ps)
                        elif t % 2 == 0:
                            nc.scalar.activation(
                                out=dst, in_=ps, func=IDENT,
                                bias=bqk_sb[:, t: t + 1], scale=1.0,
                            )
                        else:
                            nc.vector.tensor_scalar(
                                out=dst, in0=ps,
                                scalar1=bqk_sb[:, t: t + 1], scalar2=None,
                                op0=mybir.AluOpType.add,
                            )

                # ---- phase 2: v (padded 66-wide head-pair slots, ones col
                #      per head for rowsums; bias row plants the ones) ----
                for st in range(16):
                    ps = scp12.tile([128, 512], F32, tag="sc", name=f"p2_{st}")
                    for ks in range(2):
                        nc.tensor.matmul(
                            out=ps[:, 0:264],
                            lhsT=xt_sb[:, ks * S + st * 128: ks * S + st * 128 + 128],
                            rhs=wv_sb[:, ks * 264: ks * 264 + 264],
                            start=(ks == 0), stop=False,
                        )
                    nc.tensor.matmul(
                        out=ps[:, 0:264],
                        lhsT=ones_sb[0:1, 0:128],
                        rhs=bv_sb[0:1, 0:264],
                        start=False, stop=True,
                    )
                    dst = v_sb[:, st * 264: st * 264 + 264]
                    if st % 2 == 0:
                        nc.scalar.copy(out=dst, in_=ps[:, 0:264])
                    else:
                        nc.vector.tensor_copy(out=dst, in_=ps[:, 0:264])

                p12.__exit__(None, None, None)

                if DEBUG_DUMPS:
                    for ch in range(4):
                        nc.sync.dma_start(
                            out=dbg["dbg_qT"][:, ch * 1024: ch * 1024 + 1024],
                            in_=qT_sb[:, ch * 1024: ch * 1024 + 1024])
                        nc.sync.dma_start(
                            out=dbg["dbg_kT"][:, ch * 1024: ch * 1024 + 1024],
                            in_=kT_sb[:, ch * 1024: ch * 1024 + 1024])
                    for ch in range(4):
                        nc.sync.dma_start(
                            out=dbg["dbg_v"][:, ch * 1056: ch * 1056 + 1056],
                            in_=v_sb[:, ch * 1056: ch * 1056 + 1056])
                # ---- phase 3: attention, flat stream of (qb, g, kt) groups ----
                scp_cm = tc.tile_pool(name="scp", bufs=1, space="PSUM")
                scp = scp_cm.__enter__()
                ng = NQB * 32

                eT_ring = {}      # (gi//2) -> eT tile for a group pair
                blk = {}          # qb -> dict of boundary tiles

                # one persistent [128,512] bank per PE row tile position:
                # matmuls at different ROW tile positions must not write the
                # same PSUM bank (hw fault). Bank i holds head-lane i scores
                # for two consecutive groups (g=0 cols 0:256, g=1 cols
                # 256:512 of the same kt chunk).
                sc_banks = [
                    scp.tile([128, 512], F32, tag=f"sc{i}", name=f"sc{i}")
                    for i in range(4)
                ]

                def emit_scores_exp(gi):
                    qb, r = divmod(gi, 32)
                    kt, g = divmod(r, 2)
                    for i in range(4):
                        nc.tensor.matmul(
                            out=sc_banks[i][:, g * 256: g * 256 + 256],
                            lhsT=kT_sb[32 * i: 32 * i + 32,
                                       g * S + kt * 128: g * S + kt * 128 + 128],
                            rhs=qT_sb[32 * i: 32 * i + 32,
                                      g * S + qb * QBS: g * S + qb * QBS + QBS],
                            start=True, stop=True,
                            tile_position=(32 * i, 0),
                        )
                    if g == 1:  # pair complete: exp all 8 heads for this kt
                        eT = etp.tile([128, 2048], BF16, tag="eT",
                                      name=f"eT{gi}")
                        use_act = ((gi // 2) * ACT_SHARE) % 16 < ACT_SHARE
                        for i in range(4):
                            eT_h = eT[:, i * 512: i * 512 + 512]
                            if use_act:
                                nc.scalar.activation(
                                    out=eT_h, in_=sc_banks[i], func=EXP,
                                    scale=SCALE,
                                )
                            else:
                                nc.vector.tensor_scalar(
                                    out=eT_h.bitcast(I16), in0=sc_banks[i],
                                    scalar1=A_HACK, scalar2=B_HACK,
                                    op0=mybir.AluOpType.mult,
                                    op1=mybir.AluOpType.add,
                                )
                        eT_ring[gi // 2] = eT
                        if DEBUG_DUMPS and gi == 1:
                            nc.sync.dma_start(out=dbg["dbg_eT"][:, :],
                                              in_=eT[:, 0:1024])

                def emit_ctx(ci):
                    qb, r = divmod(ci, 32)
                    kt, g = divmod(r, 2)
                    eT = eT_ring[ci // 2] if g == 0 else eT_ring.pop(ci // 2)
                    ctx_ps = ctx_bufs[qb % 2]
                    for pi in range(2):
                        pair = g * 2 + pi
                        vc = kt * 264 + pair * 66
                        # start=True marks the WHOLE 2KB bank row-band as
                        # pending-zero, so only the first pair in the bank
                        # may start; the odd pair's kt=0 writes land on
                        # still-pending bytes and overwrite (= fresh start).
                        st_flag = (kt == 0 and pi == 0)
                        nc.tensor.matmul(
                            out=ctx_ps[0:33, pair * 256: pair * 256 + 256],
                            lhsT=v_sb[:, vc: vc + 33],
                            rhs=eT[:, (2 * pi) * 512 + g * 256:
                                    (2 * pi) * 512 + g * 256 + 256],
                            start=st_flag, stop=(kt == 15),
                            tile_position=(0, 0), skip_group_check=True,
                        )
                        nc.tensor.matmul(
                            out=ctx_ps[64:97, pair * 256: pair * 256 + 256],
                            lhsT=v_sb[:, vc + 33: vc + 66],
                            rhs=eT[:, (2 * pi + 1) * 512 + g * 256:
                                    (2 * pi + 1) * 512 + g * 256 + 256],
                            start=st_flag, stop=(kt == 15),
                            tile_position=(0, 64), skip_group_check=True,
                        )

                def emit_evict_recip(qb):
                    ctx_ps = ctx_bufs[qb % 2]
                    stg = stp.tile([128, 1024], BF16, tag="stg", name=f"stg{qb}")
                    nc.scalar.copy(out=stg, in_=ctx_ps)
                    if NO_RECIP:
                        blk[qb] = {"stg": stg, "rcb": None}
                        return
                    # rowsums (rows 32/96) -> DRAM rows -> [128,16] recip ->
                    # DRAM rows -> per-half broadcast
                    nc.sync.dma_start(out=rscr[0:1, :], in_=stg[32:33, :])
                    nc.sync.dma_start(out=rscr[1:2, :], in_=stg[96:97, :])
                    rsg = osb.tile([128, 16], BF16, tag="rsg", name=f"rsg{qb}")
                    nc.sync.dma_start(out=rsg, in_=rscr[:, :])
                    with nc.allow_low_precision(
                        reason="softmax rowsum recip in bf16; ~0.4% rel"
                    ):
                        nc.vector.reciprocal(out=rsg, in_=rsg)
                    nc.sync.dma_start(out=rscr2[:, :], in_=rsg)
                    rcb = stp.tile([128, 1024], BF16, tag="rcb", name=f"rcb{qb}")
                    nc.sync.dma_start(
                        out=rcb[0:64, :],
                        in_=rscr2[0:1, :].to_broadcast((64, 1024)),
                    )
                    nc.sync.dma_start(
                        out=rcb[64:128, :],
                        in_=rscr2[1:2, :].to_broadcast((64, 1024)),
                    )
                    blk[qb] = {"stg": stg, "rcb": rcb}
                    if DEBUG_DUMPS and qb == 0:
                        nc.sync.dma_start(out=dbg["dbg_stg"][:, :], in_=stg)
                        nc.sync.dma_start(out=dbg["dbg_rcb"][:, :], in_=rcb)

                def emit_norm(qb):
                    b = blk[qb]
                    ctxT = ctp.tile([128, 1024], BF16, tag="ctxT",
                                    name=f"ctxT{qb}")
                    if NO_RECIP:
                        nc.vector.tensor_copy(out=ctxT, in_=b["stg"])
                    else:
                        nc.vector.tensor_mul(out=ctxT, in0=b["stg"], in1=b["rcb"])
                    b["ctxT"] = ctxT
                    if DEBUG_DUMPS and qb == 0:
                        nc.sync.dma_start(out=dbg["dbg_ctxT"][:, :], in_=ctxT)

                def emit_outproj(qb):
                    b = blk.pop(qb)
                    ctxT = b["ctxT"]
                    ctx_ps = ctx_bufs[qb % 2]
                    po = osb.tile([128, 512], F32, tag="ot", name=f"ot{qb}")
                    for st in range(2):
                        if OUTPROJ_SPLIT:
                            for pair in range(4):
                                for cg in range(2):
                                    nc.tensor.matmul(
                                        out=ctx_ps[64 * cg: 64 * cg + 64,
                                                   st * 256: st * 256 + 256],
                                        lhsT=ctxT[:, pair * 256 + st * 128 + 64 * cg:
                                                  pair * 256 + st * 128 + 64 * cg + 64],
                                        rhs=wo_sb[:, pair * 256: pair * 256 + 256],
                                        start=(pair == 0 and cg == 0),
                                        stop=(pair == 3),
                                        tile_position=(0, 64 * cg),
                                        skip_group_check=True,
                                    )
                        else:
                            for pair in range(4):
                                nc.tensor.matmul(
                                    out=ctx_ps[:, st * 256: st * 256 + 256],
                                    lhsT=ctxT[:, pair * 256 + st * 128:
                                              pair * 256 + st * 128 + 128],
                                    rhs=wo_sb[:, pair * 256: pair * 256 + 256],
                                    start=(pair == 0), stop=(pair == 3),
                                    tile_position=(0, 0),
                                    skip_group_check=True,
                                )
                        nc.scalar.copy(
                            out=po[:, st * 256: st * 256 + 256],
                            in_=ctx_ps[:, st * 256: st * 256 + 256],
                        )
                        nc.sync.dma_start(
                            out=out[qb * QBS + st * 128: qb * QBS + st * 128 + 128, :],
                            in_=po[:, st * 256: st * 256 + 256],
                        )

                n_slots = ng + CTX_DELAY + PROJ_OFF
                if MAX_GI is not None:
                    n_slots = min(n_slots, MAX_GI)
                for gi in range(n_slots):
                    if gi < ng:
                        emit_scores_exp(gi)
                    ci = gi - CTX_DELAY
                    if 0 <= ci < ng:
                        emit_ctx(ci)
                    # staggered boundary work per finished block
                    for qb in range(NQB):
                        off = ci - (qb * 32 + 31)
                        if off == EV_OFF:
                            emit_evict_recip(qb)
                        elif off == NORM_OFF:
                            emit_norm(qb)
                        elif off == PROJ_OFF:
                            emit_outproj(qb)
                scp_cm.__exit__(None, None, None)
    if legalize:
        _legalize_sync_waits(nc)
    return nc


_NC_CACHE = None


def _get_nc():
    global _NC_CACHE
    if _NC_CACHE is None:
        _NC_CACHE = _build_nc()
    return _NC_CACHE


def _ks_layout(a, nk, cols):
    """[nk*128, cols] -> [128, nk*cols] with [p, k*cols+c] = a[k*128+p, c]."""
    return np.ascontiguousarray(
        a.reshape(nk, 128, cols).transpose(1, 0, 2).reshape(128, nk * cols)
    )


def _prep_in_maps(x, w_qkv, b_qkv, w_out, b_out):
    x = np.asarray(x, dtype=np.float32)
    w_qkv = np.asarray(w_qkv, dtype=np.float32)
    b_qkv = np.asarray(b_qkv, dtype=np.float32)
    w_out = np.asarray(w_out, dtype=np.float32)
    b_out = np.asarray(b_out, dtype=np.float32)

    wqk_l = _ks_layout(w_qkv[:, : 2 * H], 2, 512).astype(NPBF16)

    # q/k bias as per-partition vectors, one column per phase-1 tile
    bqk_t = np.ascontiguousarray(
        b_qkv[: 2 * H].reshape(4, 128).T).astype(np.float32)

    # v weights: head-pair slots of 66: [v_2p |1| v_2p+1 |1] (ones via bias row)
    wpad = np.zeros((H, 264), np.float32)
    bvr = np.zeros((1, 264), np.float32)
    for h in range(NH):
        c0 = (h // 2) * 66 + (h % 2) * 33
        wpad[:, c0: c0 + 32] = w_qkv[:, 2 * H + h * HD: 2 * H + (h + 1) * HD]
        bvr[0, c0: c0 + 32] = b_qkv[2 * H + h * HD: 2 * H + (h + 1) * HD]
        bvr[0, c0 + 32] = 1.0
    wv_l = _ks_layout(wpad, 2, 264).astype(NPBF16)

    # w_out rows permuted into ctxT slot layout: per pair-block of 128 rows:
    # [head 2p (32) | b_out row (pair 0 only) | 31 zeros | head 2p+1 (32) | 32 zeros]
    wo_perm = np.zeros((512, H), np.float32)
    for pair in range(4):
        r0 = pair * 128
        wo_perm[r0: r0 + 32, :] = w_out[(2 * pair) * HD: (2 * pair + 1) * HD, :]
        wo_perm[r0 + 64: r0 + 96, :] = w_out[(2 * pair + 1) * HD: (2 * pair + 2) * HD, :]
    wo_perm[32, :] = b_out  # ctxT row 32 of pair 0 is rs*1/rs = 1
    wo_l = _ks_layout(wo_perm, 4, 256).astype(NPBF16)

    shared = {
        "wqk": wqk_l,
        "wv": wv_l,
        "bqk": bqk_t,
        "bv": bvr.astype(NPBF16),
        "wo": wo_l,
        "ones": np.ones((1, 512), NPBF16),
    }
    in_maps = []
    for b in range(B):
        xtm = _ks_layout(np.ascontiguousarray(x[b].T), 2, S).astype(NPBF16)
        in_maps.append({"xt": xtm, **shared})
    return in_maps


def kernel(x, w_qkv, b_qkv, w_out, b_out):
    in_maps = _prep_in_maps(x, w_qkv, b_qkv, w_out, b_out)
    nc = _get_nc()
    res = run_bass_kernel_spmd(nc, in_maps, list(range(N_CORES)), **TRACE_OPTS)
    global LAST_RESULT
    LAST_RESULT = res
    return np.stack([res.results[b]["out"] for b in range(B)], axis=0)


# revision 18
# speedup vs baseline: 1.4226x; 1.4226x over previous
"""Multi-head self-attention (B=8, S=2048, H=256, NH=8, HD=32) on 8 TRN2 cores.

v4: data-parallel over batch (1 batch element/core). Lessons baked in:
  - PE streams at ~0.42ns/col when matmuls are >=512 cols; 256-col
    matmuls can't hide LDWEIGHTS (~100-150ns) -> keep 512-col streams
    (4 q-blocks of 512, v2's shapes).
  - matmuls at different PE ROW tile positions must never write the
    same PSUM bank (hw fault): scores use 4 position-locked banks.
  - flat global group stream; ctx trails scores by CTX_DELAY groups so
    block-boundary work hides behind queued PE work; boundary emissions
    staggered (evict+recip / normalize / out-proj) to avoid in-order
    engine stalls.
  - boundary chain in bf16: stg evict bf16 (rowsums ride along), bf16
    recip roundtrip, bf16 broadcast, 2x-mode DVE normalize.
  - out-projection as single 128-wide stationary tiles (16384 cols vs
    v2's 32768); q/k bias folded into phase-1 evictions as per-partition
    vectors (drops 16 rank-1 matmuls).
"""
import numpy as np
import ml_dtypes

import bass_rust
import concourse.bass as bass
import concourse.mybir as mybir
import concourse.tile as tile
from concourse.bass_utils import run_bass_kernel_spmd

BF16 = mybir.dt.bfloat16
F32 = mybir.dt.float32
I16 = mybir.dt.int16
NPBF16 = ml_dtypes.bfloat16

B, S, H = 8, 2048, 256
NH, HD = 8, 32
SCALE = 1.0 / float(np.sqrt(HD))
N_CORES = 8

LOG2E = 1.4426950408889634
# DVE bit-hack exp constants (c centers the mantissa-interp sawtooth; the
# +0.5 assumes truncation on float->int convert)
HACK_C = 5.5
HACK_ROUND = True
A_HACK = float(LOG2E * 128.0 * SCALE)
B_HACK = float(127.0 * 128.0 - HACK_C + (0.0 if HACK_ROUND else 0.5))

NQB = 4          # q-blocks
QBS = 512        # q-block size
CTX_DELAY = 6    # groups between a group's scores and its ctx matmuls
EV_OFF = 1       # boundary stagger (slots after a block's last ctx batch)
NORM_OFF = 4
PROJ_OFF = 6
ACT_SHARE = 8    # of every 16 exp units, this many go to ScalarE
MAX_GI = None    # truncate slot loop (bisect)

TRACE_OPTS = {}
LAST_RESULT = None


def _legalize_sync_waits(nc):
    """Split multi-wait sync_infos onto NoOp carriers (walrus allows 1/inst)."""
    n = 0
    for f in nc.m.functions:
        for bb in f.blocks:
            insts = bb.instructions
            i = 0
            while i < len(insts):
                inst = insts[i]
                si = inst.sync_info
                if si is not None and len(si.on_wait) > 1:
                    waits = list(si.on_wait)
                    carriers = []
                    for w in waits[:-1]:
                        carriers.append(
                            mybir.InstNoOp(
                                name=f"{inst.name}-w{n}",
                                sync_info=mybir.SyncInfo(on_wait=[w], on_update=[]),
                                bass_nofuse=True,
                                engine=inst.engine,
                            )
                        )
                        n += 1
                    inst.sync_info = bass_rust.SyncInfo(
                        on_wait=waits[-1:], on_update=list(si.on_update)
                    )
                    insts[i:i] = carriers
                    i += len(carriers)
                i += 1
    return n


def _build_nc(legalize=True):
    nc = bass.Bass()
    xt = nc.dram_tensor("xt", [128, 2 * S], BF16, kind="ExternalInput")
    wqk = nc.dram_tensor("wqk", [128, 2 * 512], BF16, kind="ExternalInput")
    bqk = nc.dram_tensor("bqk", [128, 4], F32, kind="ExternalInput")
    bv = nc.dram_tensor("bv", [1, 264], BF16, kind="ExternalInput")
    wv = nc.dram_tensor("wv", [128, 2 * 264], BF16, kind="ExternalInput")
    wo = nc.dram_tensor("wo", [128, 4 * 256], BF16, kind="ExternalInput")
    ones = nc.dram_tensor("ones", [1, 512], BF16, kind="ExternalInput")
    out = nc.dram_tensor("out", [S, H], F32, kind="ExternalOutput")
    # rowsum gather / reciprocal round-trip scratch ([2,2048] <-> [128,32])
    rscr = nc.dram_tensor("rscr", [2, 2048], BF16)
    rscr2 = nc.dram_tensor("rscr2", [2, 2048], BF16)

    EXP = mybir.ActivationFunctionType.Exp
    IDENT = mybir.ActivationFunctionType.Identity

    with tile.TileContext(nc) as tc:
        with (
            tc.tile_pool(name="const", bufs=1) as const,
            tc.tile_pool(name="etp", bufs=15) as etp,
            tc.tile_pool(name="ctp", bufs=2) as ctp,
            tc.tile_pool(name="stp", bufs=2) as stp,
            tc.tile_pool(name="osb", bufs=2) as osb,
        ):
            # ---- input DMAs, in first-use order ----
            wqk_sb = const.tile([128, 2 * 512], BF16, tag="wqk")
            nc.sync.dma_start(out=wqk_sb, in_=wqk[:, :])
            xt_sb = const.tile([128, 2 * S], BF16, tag="xt")
            for ch in (0, 2, 1, 3):  # phase 1 consumes ks-paired halves
                nc.sync.dma_start(
                    out=xt_sb[:, ch * 1024: ch * 1024 + 1024],
                    in_=xt[:, ch * 1024: ch * 1024 + 1024])
            bqk_sb = const.tile([128, 4], F32, tag="bqk")
            nc.sync.dma_start(out=bqk_sb, in_=bqk[:, :])
            wv_sb = const.tile([128, 2 * 264], BF16, tag="wv")
            nc.sync.dma_start(out=wv_sb, in_=wv[:, :])
            bv_sb = const.tile([1, 264], BF16, tag="bv")
            nc.sync.dma_start(out=bv_sb, in_=bv[:, :])
            ones_sb = const.tile([1, 512], BF16, tag="ones")
            nc.sync.dma_start(out=ones_sb, in_=ones[:, :])
            wo_sb = const.tile([128, 4 * 256], BF16, tag="wo")
            nc.sync.dma_start(out=wo_sb, in_=wo[:, :])

            qT_sb = const.tile([128, 2 * S], BF16, tag="qT")
            kT_sb = const.tile([128, 2 * S], BF16, tag="kT")
            v_sb = const.tile([128, 16 * 264], BF16, tag="v")

            # persistent PSUM: ctx accumulators (4 banks, one per pair)
            with (
                tc.tile_pool(name="cxp", bufs=1, space="PSUM") as cxp,
            ):
                ctx_ps = cxp.tile([128, 2048], F32, tag="ctx", name="ctx")

                # ---- warmup (p-state ramp) while input DMAs land ----
                warm_sb = const.tile([128, 512], BF16, tag="warm")
                nc.vector.memset(warm_sb, 0.0)
                for r in range(12):
                    nc.tensor.matmul(
                        out=ctx_ps[:, 0:512], lhsT=warm_sb[:, 0:128],
                        rhs=warm_sb[:, :], start=True, stop=True,
                    )
                # zero the never-matmul-written ctx rows so full-height
                # eviction reads finite values (persist across q-blocks)
                nc.vector.memset(ctx_ps[32:64, :], 0.0)
                nc.vector.memset(ctx_ps[96:128, :], 0.0)

                # ---- phase 1: qT/kT [feature, s]; bias folded into the
                #      evictions as per-partition vectors ----
                p12 = tc.tile_pool(name="p12", bufs=4, space="PSUM")
                scp12 = p12.__enter__()
                for nb in range(4):  # s blocks of 512
                    for t in range(4):  # feature tiles: q0,q1,k0,k1
                        ps = scp12.tile([128, 512], F32, tag="sc",
                                        name=f"p1_{t}_{nb}")
                        for ks in range(2):
                            nc.tensor.matmul(
                                out=ps,
                                lhsT=wqk_sb[:, ks * 512 + t * 128: ks * 512 + t * 128 + 128],
                                rhs=xt_sb[:, ks * S + nb * 512: ks * S + nb * 512 + 512],
                                start=(ks == 0), stop=(ks == 1),
                            )
                        dst = (qT_sb if t < 2 else kT_sb)[
                            :, (t % 2) * S + nb * 512: (t % 2) * S + nb * 512 + 512
                        ]
                        if t % 2 == 0:
                            nc.scalar.activation(
                                out=dst, in_=ps, func=IDENT,
                                bias=bqk_sb[:, t: t + 1], scale=1.0,
                            )
                        else:
                            nc.vector.tensor_scalar(
                                out=dst, in0=ps,
                                scalar1=bqk_sb[:, t: t + 1], scalar2=None,
                                op0=mybir.AluOpType.add,
                            )

                # ---- phase 2: v (padded 66-wide head-pair slots, ones col
                #      per head for rowsums; bias row plants the ones) ----
                for st in range(16):
                    ps = scp12.tile([128, 512], F32, tag="sc", name=f"p2_{st}")
                    for ks in range(2):
                        nc.tensor.matmul(
                            out=ps[:, 0:264],
                            lhsT=xt_sb[:, ks * S + st * 128: ks * S + st * 128 + 128],
                            rhs=wv_sb[:, ks * 264: ks * 264 + 264],
                            start=(ks == 0), stop=False,
                        )
                    nc.tensor.matmul(
                        out=ps[:, 0:264],
                        lhsT=ones_sb[0:1, 0:128],
                        rhs=bv_sb[0:1, 0:264],
                        start=False, stop=True,
                    )
                    dst = v_sb[:, st * 264: st * 264 + 264]
                    if st % 2 == 0:
                        nc.scalar.copy(out=dst, in_=ps[:, 0:264])
                    else:
                        nc.vector.tensor_copy(out=dst, in_=ps[:, 0:264])

                p12.__exit__(None, None, None)

                # ---- phase 3: attention, flat stream of (qb, g, kt) groups ----
                scp_cm = tc.tile_pool(name="scp", bufs=1, space="PSUM")
                scp = scp_cm.__enter__()
                ng = NQB * 32

                eT_ring = {}      # gi -> eT tile awaiting ctx
                blk = {}          # qb -> dict of boundary tiles

                # one persistent [128,512] bank per PE row tile position:
                # matmuls at different ROW tile positions must not write the
                # same PSUM bank (hw fault).
                sc_banks = [
                    scp.tile([128, 512], F32, tag=f"sc{i}", name=f"sc{i}")
                    for i in range(4)
                ]

                def emit_scores_exp(gi):
                    qb, r = divmod(gi, 32)
                    kt, g = divmod(r, 2)
                    eT = etp.tile([128, 2048], BF16, tag="eT", name=f"eT{gi}")
                    for i in range(4):
                        nc.tensor.matmul(
                            out=sc_banks[i],
                            lhsT=kT_sb[32 * i: 32 * i + 32,
                                       g * S + kt * 128: g * S + kt * 128 + 128],
                            rhs=qT_sb[32 * i: 32 * i + 32,
                                      g * S + qb * QBS: g * S + qb * QBS + QBS],
                            start=True, stop=True,
                            tile_position=(32 * i, 0),
                        )
                        eT_h = eT[:, i * 512: i * 512 + 512]
                        if ((gi * 4 + i) * ACT_SHARE) % 16 < ACT_SHARE:
                            nc.scalar.activation(
                                out=eT_h, in_=sc_banks[i], func=EXP,
                                scale=SCALE,
                            )
                        else:
                            nc.vector.tensor_scalar(
                                out=eT_h.bitcast(I16), in0=sc_banks[i],
                                scalar1=A_HACK, scalar2=B_HACK,
                                op0=mybir.AluOpType.mult,
                                op1=mybir.AluOpType.add,
                            )
                    eT_ring[gi] = eT

                def emit_ctx(ci):
                    qb, r = divmod(ci, 32)
                    kt, g = divmod(r, 2)
                    eT = eT_ring.pop(ci)
                    for pi in range(2):
                        pair = g * 2 + pi
                        vc = kt * 264 + pair * 66
                        nc.tensor.matmul(
                            out=ctx_ps[0:33, pair * 512: pair * 512 + 512],
                            lhsT=v_sb[:, vc: vc + 33],
                            rhs=eT[:, (2 * pi) * 512: (2 * pi) * 512 + 512],
                            start=(kt == 0), stop=(kt == 15),
                            tile_position=(0, 0), skip_group_check=True,
                        )
                        nc.tensor.matmul(
                            out=ctx_ps[64:97, pair * 512: pair * 512 + 512],
                            lhsT=v_sb[:, vc + 33: vc + 66],
                            rhs=eT[:, (2 * pi + 1) * 512: (2 * pi + 1) * 512 + 512],
                            start=(kt == 0), stop=(kt == 15),
                            tile_position=(0, 64), skip_group_check=True,
                        )

                def emit_evict_recip(qb):
                    stg = stp.tile([128, 2048], BF16, tag="stg", name=f"stg{qb}")
                    nc.scalar.copy(out=stg, in_=ctx_ps)
                    # rowsums (rows 32/96) -> DRAM rows -> [128,32] recip ->
                    # DRAM rows -> per-half broadcast
                    nc.sync.dma_start(out=rscr[0:1, :], in_=stg[32:33, :])
                    nc.sync.dma_start(out=rscr[1:2, :], in_=stg[96:97, :])
                    rsg = osb.tile([128, 32], BF16, tag="rsg", name=f"rsg{qb}")
                    nc.sync.dma_start(out=rsg, in_=rscr[:, :])
                    with nc.allow_low_precision(
                        reason="softmax rowsum recip in bf16; ~0.4% rel"
                    ):
                        nc.vector.reciprocal(out=rsg, in_=rsg)
                    nc.sync.dma_start(out=rscr2[:, :], in_=rsg)
                    rcb = stp.tile([128, 2048], BF16, tag="rcb", name=f"rcb{qb}")
                    nc.sync.dma_start(
                        out=rcb[0:64, :],
                        in_=rscr2[0:1, :].to_broadcast((64, 2048)),
                    )
                    nc.sync.dma_start(
                        out=rcb[64:128, :],
                        in_=rscr2[1:2, :].to_broadcast((64, 2048)),
                    )
                    blk[qb] = {"stg": stg, "rcb": rcb}

                def emit_norm(qb):
                    b = blk[qb]
                    ctxT = ctp.tile([128, 2048], BF16, tag="ctxT",
                                    name=f"ctxT{qb}")
                    nc.vector.tensor_mul(out=ctxT, in0=b["stg"], in1=b["rcb"])
                    b["ctxT"] = ctxT

                def emit_outproj(qb):
                    b = blk.pop(qb)
                    ctxT = b["ctxT"]
                    po = osb.tile([128, 1024], F32, tag="ot", name=f"ot{qb}")
                    for st in range(4):
                        for pair in range(4):
                            nc.tensor.matmul(
                                out=ctx_ps[:, st * 512: st * 512 + 256],
                                lhsT=ctxT[:, pair * 512 + st * 128:
                                          pair * 512 + st * 128 + 128],
                                rhs=wo_sb[:, pair * 256: pair * 256 + 256],
                                start=(pair == 0), stop=(pair == 3),
                                tile_position=(0, 0),
                                skip_group_check=True,
                            )
                        nc.scalar.copy(
                            out=po[:, st * 256: st * 256 + 256],
                            in_=ctx_ps[:, st * 512: st * 512 + 256],
                        )
                        nc.sync.dma_start(
                            out=out[qb * QBS + st * 128: qb * QBS + st * 128 + 128, :],
                            in_=po[:, st * 256: st * 256 + 256],
                        )

                # ctx schedule: each block's first ctx batches are delayed
                # by D extra slots (scores keep the PE busy meanwhile) so the
                # PREVIOUS block's out-proj can finish with the ctx banks
                # free; the backlog catches up at 2 ctx batches per slot.
                D = 7

                def ctx_slot(ci):
                    qb, l = divmod(ci, 32)
                    if l < 2 * D:
                        return qb * 32 + CTX_DELAY + D + (l + 1) // 2
                    return ci + CTX_DELAY

                ctx_by_slot = {}
                for ci in range(ng):
                    ctx_by_slot.setdefault(ctx_slot(ci), []).append(ci)
                last_emit = {qb: ctx_slot(qb * 32 + 31) for qb in range(NQB)}
                n_slots = (max(last_emit.values()) + PROJ_OFF + 1) if ng else 0
                if MAX_GI is not None:
                    n_slots = min(n_slots, MAX_GI)
                for gi in range(n_slots):
                    if gi < ng:
                        emit_scores_exp(gi)
                    for ci in ctx_by_slot.get(gi, []):
                        emit_ctx(ci)
                    # staggered boundary work per finished block
                    for qb in range(NQB):
                        base = last_emit[qb]
                        if gi == base + EV_OFF:
                            emit_evict_recip(qb)
                        elif gi == base + NORM_OFF:
                            emit_norm(qb)
                        elif gi == base + PROJ_OFF:
                            emit_outproj(qb)
                scp_cm.__exit__(None, None, None)
    if legalize:
        _legalize_sync_waits(nc)
    return nc


_NC_CACHE = None


def _get_nc():
    global _NC_CACHE
    if _NC_CACHE is None:
        _NC_CACHE = _build_nc()
    return _NC_CACHE


def _ks_layout(a, nk, cols):
    """[nk*128, cols] -> [128, nk*cols] with [p, k*cols+c] = a[k*128+p, c]."""
    return np.ascontiguousarray(
        a.reshape(nk, 128, cols).transpose(1, 0, 2).reshape(128, nk * cols)
    )


def _prep_in_maps(x, w_qkv, b_qkv, w_out, b_out):
    x = np.asarray(x, dtype=np.float32)
    w_qkv = np.asarray(w_qkv, dtype=np.float32)
    b_qkv = np.asarray(b_qkv, dtype=np.float32)
    w_out = np.asarray(w_out, dtype=np.float32)
    b_out = np.asarray(b_out, dtype=np.float32)

    wqk_l = _ks_layout(w_qkv[:, : 2 * H], 2, 512).astype(NPBF16)

    # q/k bias as per-partition vectors, one column per phase-1 tile
    bqk_t = np.ascontiguousarray(
        b_qkv[: 2 * H].reshape(4, 128).T).astype(np.float32)

    # v weights: head-pair slots of 66: [v_2p |1| v_2p+1 |1] (ones via bias row)
    wpad = np.zeros((H, 264), np.float32)
    bvr = np.zeros((1, 264), np.float32)
    for h in range(NH):
        c0 = (h // 2) * 66 + (h % 2) * 33
        wpad[:, c0: c0 + 32] = w_qkv[:, 2 * H + h * HD: 2 * H + (h + 1) * HD]
        bvr[0, c0: c0 + 32] = b_qkv[2 * H + h * HD: 2 * H + (h + 1) * HD]
        bvr[0, c0 + 32] = 1.0
    wv_l = _ks_layout(wpad, 2, 264).astype(NPBF16)

    # w_out rows permuted into ctxT slot layout: per pair-block of 128 rows:
    # [head 2p (32) | b_out row (pair 0 only) | 31 zeros | head 2p+1 (32) | 32 zeros]
    wo_perm = np.zeros((512, H), np.float32)
    for pair in range(4):
        r0 = pair * 128
        wo_perm[r0: r0 + 32, :] = w_out[(2 * pair) * HD: (2 * pair + 1) * HD, :]
        wo_perm[r0 + 64: r0 + 96, :] = w_out[(2 * pair + 1) * HD: (2 * pair + 2) * HD, :]
    wo_perm[32, :] = b_out  # ctxT row 32 of pair 0 is rs*1/rs = 1
    wo_l = _ks_layout(wo_perm, 4, 256).astype(NPBF16)

    shared = {
        "wqk": wqk_l,
        "wv": wv_l,
        "bqk": bqk_t,
        "bv": bvr.astype(NPBF16),
        "wo": wo_l,
        "ones": np.ones((1, 512), NPBF16),
    }
    in_maps = []
    for b in range(B):
        xtm = _ks_layout(np.ascontiguousarray(x[b].T), 2, S).astype(NPBF16)
        in_maps.append({"xt": xtm, **shared})
    return in_maps


def kernel(x, w_qkv, b_qkv, w_out, b_out):
    in_maps = _prep_in_maps(x, w_qkv, b_qkv, w_out, b_out)
    nc = _get_nc()
    res = run_bass_kernel_spmd(nc, in_maps, list(range(N_CORES)), **TRACE_OPTS)
    global LAST_RESULT
    LAST_RESULT = res
    return np.stack([res.results[b]["out"] for b in range(B)], axis=0)


# revision 20
# speedup vs baseline: 1.4834x; 1.0428x over previous
"""Multi-head self-attention (B=8, S=2048, H=256, NH=8, HD=32) on 8 TRN2 cores.

v4: data-parallel over batch (1 batch element/core). Lessons baked in:
  - PE streams at ~0.42ns/col when matmuls are >=512 cols; 256-col
    matmuls can't hide LDWEIGHTS (~100-150ns) -> keep 512-col streams
    (4 q-blocks of 512, v2's shapes).
  - matmuls at different PE ROW tile positions must never write the
    same PSUM bank (hw fault): scores use 4 position-locked banks.
  - flat global group stream; ctx trails scores by CTX_DELAY groups so
    block-boundary work hides behind queued PE work; boundary emissions
    staggered (evict+recip / normalize / out-proj) to avoid in-order
    engine stalls.
  - boundary chain in bf16: stg evict bf16 (rowsums ride along), bf16
    recip roundtrip, bf16 broadcast, 2x-mode DVE normalize.
  - out-projection as single 128-wide stationary tiles (16384 cols vs
    v2's 32768); q/k bias folded into phase-1 evictions as per-partition
    vectors (drops 16 rank-1 matmuls).
"""
import numpy as np
import ml_dtypes

import bass_rust
import concourse.bass as bass
import concourse.mybir as mybir
import concourse.tile as tile
from concourse.bass_utils import run_bass_kernel_spmd

BF16 = mybir.dt.bfloat16
F32 = mybir.dt.float32
I16 = mybir.dt.int16
NPBF16 = ml_dtypes.bfloat16

B, S, H = 8, 2048, 256
NH, HD = 8, 32
SCALE = 1.0 / float(np.sqrt(HD))
N_CORES = 8

LOG2E = 1.4426950408889634
# DVE bit-hack exp constants (c centers the mantissa-interp sawtooth; the
# +0.5 assumes truncation on float->int convert)
HACK_C = 5.5
HACK_ROUND = True
A_HACK = float(LOG2E * 128.0 * SCALE)
B_HACK = float(127.0 * 128.0 - HACK_C + (0.0 if HACK_ROUND else 0.5))

NQB = 4          # q-blocks
QBS = 512        # q-block size
CTX_DELAY = 6    # groups between a group's scores and its ctx matmuls
EV_OFF = 1       # boundary stagger (slots after a block's last ctx batch)
NORM_OFF = 4
PROJ_OFF = 6
ACT_SHARE = 8    # of every 16 exp units, this many go to ScalarE
MAX_GI = None    # truncate slot loop (bisect)

TRACE_OPTS = {}
LAST_RESULT = None


def _legalize_sync_waits(nc):
    """Split multi-wait sync_infos onto NoOp carriers (walrus allows 1/inst)."""
    n = 0
    for f in nc.m.functions:
        for bb in f.blocks:
            insts = bb.instructions
            i = 0
            while i < len(insts):
                inst = insts[i]
                si = inst.sync_info
                if si is not None and len(si.on_wait) > 1:
                    waits = list(si.on_wait)
                    carriers = []
                    for w in waits[:-1]:
                        carriers.append(
                            mybir.InstNoOp(
                                name=f"{inst.name}-w{n}",
                                sync_info=mybir.SyncInfo(on_wait=[w], on_update=[]),
                                bass_nofuse=True,
                                engine=inst.engine,
                            )
                        )
                        n += 1
                    inst.sync_info = bass_rust.SyncInfo(
                        on_wait=waits[-1:], on_update=list(si.on_update)
                    )
                    insts[i:i] = carriers
                    i += len(carriers)
                i += 1
    return n


def _build_nc(legalize=True):
    nc = bass.Bass()
    xt = nc.dram_tensor("xt", [128, 2 * S], BF16, kind="ExternalInput")
    wqk = nc.dram_tensor("wqk", [128, 2 * 512], BF16, kind="ExternalInput")
    bqk = nc.dram_tensor("bqk", [128, 4], F32, kind="ExternalInput")
    bv = nc.dram_tensor("bv", [1, 264], BF16, kind="ExternalInput")
    wv = nc.dram_tensor("wv", [128, 2 * 264], BF16, kind="ExternalInput")
    wo = nc.dram_tensor("wo", [128, 4 * 256], BF16, kind="ExternalInput")
    ones = nc.dram_tensor("ones", [1, 512], BF16, kind="ExternalInput")
    out = nc.dram_tensor("out", [S, H], F32, kind="ExternalOutput")
    # rowsum gather / reciprocal round-trip scratch ([2,2048] <-> [128,32])
    rscr = nc.dram_tensor("rscr", [2, 2048], BF16)
    rscr2 = nc.dram_tensor("rscr2", [2, 2048], BF16)

    EXP = mybir.ActivationFunctionType.Exp
    IDENT = mybir.ActivationFunctionType.Identity

    with tile.TileContext(nc) as tc:
        with (
            tc.tile_pool(name="const", bufs=1) as const,
            tc.tile_pool(name="etp", bufs=15) as etp,
            tc.tile_pool(name="ctp", bufs=2) as ctp,
            tc.tile_pool(name="stp", bufs=2) as stp,
            tc.tile_pool(name="osb", bufs=2) as osb,
        ):
            # ---- input DMAs, in first-use order ----
            wqk_sb = const.tile([128, 2 * 512], BF16, tag="wqk")
            nc.sync.dma_start(out=wqk_sb, in_=wqk[:, :])
            xt_sb = const.tile([128, 2 * S], BF16, tag="xt")
            for ch in (0, 2, 1, 3):  # phase 1 consumes ks-paired halves
                nc.sync.dma_start(
                    out=xt_sb[:, ch * 1024: ch * 1024 + 1024],
                    in_=xt[:, ch * 1024: ch * 1024 + 1024])
            bqk_sb = const.tile([128, 4], F32, tag="bqk")
            nc.sync.dma_start(out=bqk_sb, in_=bqk[:, :])
            wv_sb = const.tile([128, 2 * 264], BF16, tag="wv")
            nc.sync.dma_start(out=wv_sb, in_=wv[:, :])
            bv_sb = const.tile([1, 264], BF16, tag="bv")
            nc.sync.dma_start(out=bv_sb, in_=bv[:, :])
            ones_sb = const.tile([1, 512], BF16, tag="ones")
            nc.sync.dma_start(out=ones_sb, in_=ones[:, :])
            wo_sb = const.tile([128, 4 * 256], BF16, tag="wo")
            nc.sync.dma_start(out=wo_sb, in_=wo[:, :])

            qT_sb = const.tile([128, 2 * S], BF16, tag="qT")
            kT_sb = const.tile([128, 2 * S], BF16, tag="kT")
            v_sb = const.tile([128, 16 * 264], BF16, tag="v")

            # persistent PSUM: ctx accumulators (4 banks, one per pair)
            with (
                tc.tile_pool(name="cxp", bufs=1, space="PSUM") as cxp,
            ):
                ctx_ps = cxp.tile([128, 2048], F32, tag="ctx", name="ctx")

                # ---- warmup (p-state ramp) while input DMAs land ----
                warm_sb = const.tile([128, 512], BF16, tag="warm")
                nc.vector.memset(warm_sb, 0.0)
                for r in range(12):
                    nc.tensor.matmul(
                        out=ctx_ps[:, 0:512], lhsT=warm_sb[:, 0:128],
                        rhs=warm_sb[:, :], start=True, stop=True,
                    )
                # zero the never-matmul-written ctx rows so full-height
                # eviction reads finite values (persist across q-blocks)
                nc.vector.memset(ctx_ps[32:64, :], 0.0)
                nc.vector.memset(ctx_ps[96:128, :], 0.0)

                # ---- phase 1: qT/kT [feature, s]; bias folded into the
                #      evictions as per-partition vectors ----
                p12 = tc.tile_pool(name="p12", bufs=4, space="PSUM")
                scp12 = p12.__enter__()
                for nb in range(4):  # s blocks of 512
                    for t in range(4):  # feature tiles: q0,q1,k0,k1
                        ps = scp12.tile([128, 512], F32, tag="sc",
                                        name=f"p1_{t}_{nb}")
                        for ks in range(2):
                            nc.tensor.matmul(
                                out=ps,
                                lhsT=wqk_sb[:, ks * 512 + t * 128: ks * 512 + t * 128 + 128],
                                rhs=xt_sb[:, ks * S + nb * 512: ks * S + nb * 512 + 512],
                                start=(ks == 0), stop=(ks == 1),
                            )
                        dst = (qT_sb if t < 2 else kT_sb)[
                            :, (t % 2) * S + nb * 512: (t % 2) * S + nb * 512 + 512
                        ]
                        if t % 2 == 0:
                            nc.scalar.activation(
                                out=dst, in_=ps, func=IDENT,
                                bias=bqk_sb[:, t: t + 1], scale=1.0,
                            )
                        else:
                            nc.vector.tensor_scalar(
                                out=dst, in0=ps,
                                scalar1=bqk_sb[:, t: t + 1], scalar2=None,
                                op0=mybir.AluOpType.add,
                            )

                # ---- phase 2: v (padded 66-wide head-pair slots, ones col
                #      per head for rowsums; bias row plants the ones) ----
                for st in range(16):
                    ps = scp12.tile([128, 512], F32, tag="sc", name=f"p2_{st}")
                    for ks in range(2):
                        nc.tensor.matmul(
                            out=ps[:, 0:264],
                            lhsT=xt_sb[:, ks * S + st * 128: ks * S + st * 128 + 128],
                            rhs=wv_sb[:, ks * 264: ks * 264 + 264],
                            start=(ks == 0), stop=False,
                        )
                    nc.tensor.matmul(
                        out=ps[:, 0:264],
                        lhsT=ones_sb[0:1, 0:128],
                        rhs=bv_sb[0:1, 0:264],
                        start=False, stop=True,
                    )
                    dst = v_sb[:, st * 264: st * 264 + 264]
                    if st % 2 == 0:
                        nc.scalar.copy(out=dst, in_=ps[:, 0:264])
                    else:
                        nc.vector.tensor_copy(out=dst, in_=ps[:, 0:264])

                p12.__exit__(None, None, None)

                # ---- phase 3: attention, flat stream of (qb, g, kt) groups ----
                scp_cm = tc.tile_pool(name="scp", bufs=1, space="PSUM")
                scp = scp_cm.__enter__()
                ng = NQB * 32

                eT_ring = {}      # gi -> eT tile awaiting ctx
                blk = {}          # qb -> dict of boundary tiles

                # one persistent [128,512] bank per PE row tile position:
                # matmuls at different ROW tile positions must not write the
                # same PSUM bank (hw fault).
                sc_banks = [
                    scp.tile([128, 512], F32, tag=f"sc{i}", name=f"sc{i}")
                    for i in range(4)
                ]

                def emit_scores_exp(gi):
                    qb, r = divmod(gi, 32)
                    kt, g = divmod(r, 2)
                    eT = etp.tile([128, 2048], BF16, tag="eT", name=f"eT{gi}")
                    for i in range(4):
                        nc.tensor.matmul(
                            out=sc_banks[i],
                            lhsT=kT_sb[32 * i: 32 * i + 32,
                                       g * S + kt * 128: g * S + kt * 128 + 128],
                            rhs=qT_sb[32 * i: 32 * i + 32,
                                      g * S + qb * QBS: g * S + qb * QBS + QBS],
                            start=True, stop=True,
                            tile_position=(32 * i, 0),
                        )
                        eT_h = eT[:, i * 512: i * 512 + 512]
                        tail = gi >= NQB * 32 - 3
                        if not tail and ((gi * 4 + i) * ACT_SHARE) % 16 < ACT_SHARE:
                            nc.scalar.activation(
                                out=eT_h, in_=sc_banks[i], func=EXP,
                                scale=SCALE,
                            )
                        else:
                            nc.vector.tensor_scalar(
                                out=eT_h.bitcast(I16), in0=sc_banks[i],
                                scalar1=A_HACK, scalar2=B_HACK,
                                op0=mybir.AluOpType.mult,
                                op1=mybir.AluOpType.add,
                            )
                    eT_ring[gi] = eT

                def emit_ctx(ci):
                    qb, r = divmod(ci, 32)
                    kt, g = divmod(r, 2)
                    eT = eT_ring.pop(ci)
                    for pi in range(2):
                        pair = g * 2 + pi
                        vc = kt * 264 + pair * 66
                        nc.tensor.matmul(
                            out=ctx_ps[0:33, pair * 512: pair * 512 + 512],
                            lhsT=v_sb[:, vc: vc + 33],
                            rhs=eT[:, (2 * pi) * 512: (2 * pi) * 512 + 512],
                            start=(kt == 0), stop=(kt == 15),
                            tile_position=(0, 0), skip_group_check=True,
                        )
                        nc.tensor.matmul(
                            out=ctx_ps[64:97, pair * 512: pair * 512 + 512],
                            lhsT=v_sb[:, vc + 33: vc + 66],
                            rhs=eT[:, (2 * pi + 1) * 512: (2 * pi + 1) * 512 + 512],
                            start=(kt == 0), stop=(kt == 15),
                            tile_position=(0, 64), skip_group_check=True,
                        )

                def emit_evict_recip(qb):
                    stg = stp.tile([128, 2048], BF16, tag="stg", name=f"stg{qb}")
                    nc.scalar.copy(out=stg, in_=ctx_ps)
                    # rowsums (rows 32/96) -> [128,32] transpose via direct
                    # SBUF->SBUF DMA -> recip -> DRAM rows -> per-half bcast
                    rsg = osb.tile([128, 32], BF16, tag="rsg", name=f"rsg{qb}")
                    nc.sync.dma_start(out=rsg[0:64, :], in_=stg[32:33, :])
                    nc.sync.dma_start(out=rsg[64:128, :], in_=stg[96:97, :])
                    with nc.allow_low_precision(
                        reason="softmax rowsum recip in bf16; ~0.4% rel"
                    ):
                        nc.vector.reciprocal(out=rsg, in_=rsg)
                    nc.sync.dma_start(out=rscr2[:, :], in_=rsg)
                    rcb = stp.tile([128, 2048], BF16, tag="rcb", name=f"rcb{qb}")
                    nc.sync.dma_start(
                        out=rcb[0:64, :],
                        in_=rscr2[0:1, :].to_broadcast((64, 2048)),
                    )
                    nc.sync.dma_start(
                        out=rcb[64:128, :],
                        in_=rscr2[1:2, :].to_broadcast((64, 2048)),
                    )
                    blk[qb] = {"stg": stg, "rcb": rcb}

                def emit_norm(qb, sts):
                    b = blk[qb]
                    if "ctxT" not in b:
                        b["ctxT"] = ctp.tile([128, 2048], BF16, tag="ctxT",
                                             name=f"ctxT{qb}")
                    def chunk(t, st):
                        # cols {pair*512 + st*128 .. +128, pair=0..4}
                        return t.rearrange("p (a b) -> p a b", a=4)[
                            :, :, st * 128: st * 128 + 128]
                    for st in sts:
                        nc.vector.tensor_mul(
                            out=chunk(b["ctxT"][:, :], st),
                            in0=chunk(b["stg"][:, :], st),
                            in1=chunk(b["rcb"][:, :], st),
                        )

                def emit_outproj(qb, sts, final=False):
                    b = blk[qb]
                    ctxT = b["ctxT"]
                    if "po" not in b:
                        b["po"] = osb.tile([128, 1024], F32, tag="ot",
                                           name=f"ot{qb}")
                    po = b["po"]
                    if final:
                        blk.pop(qb)
                    for st in sts:
                        for pair in range(4):
                            nc.tensor.matmul(
                                out=ctx_ps[:, st * 512: st * 512 + 256],
                                lhsT=ctxT[:, pair * 512 + st * 128:
                                          pair * 512 + st * 128 + 128],
                                rhs=wo_sb[:, pair * 256: pair * 256 + 256],
                                start=(pair == 0), stop=(pair == 3),
                                tile_position=(0, 0),
                                skip_group_check=True,
                            )
                        nc.scalar.copy(
                            out=po[:, st * 256: st * 256 + 256],
                            in_=ctx_ps[:, st * 512: st * 512 + 256],
                        )
                        nc.sync.dma_start(
                            out=out[qb * QBS + st * 128: qb * QBS + st * 128 + 128, :],
                            in_=po[:, st * 256: st * 256 + 256],
                        )

                # ctx schedule: each block's first ctx batches are delayed
                # by D extra slots (scores keep the PE busy meanwhile) so the
                # PREVIOUS block's out-proj can finish with the ctx banks
                # free; the backlog catches up at 2 ctx batches per slot.
                D = 7

                def ctx_slot(ci):
                    qb, l = divmod(ci, 32)
                    if l < 2 * D:
                        return qb * 32 + CTX_DELAY + D + (l + 1) // 2
                    return ci + CTX_DELAY

                ctx_by_slot = {}
                for ci in range(ng):
                    ctx_by_slot.setdefault(ctx_slot(ci), []).append(ci)
                last_emit = {qb: ctx_slot(qb * 32 + 31) for qb in range(NQB)}
                n_slots = (max(last_emit.values()) + PROJ_OFF + 1) if ng else 0
                if MAX_GI is not None:
                    n_slots = min(n_slots, MAX_GI)
                for gi in range(n_slots):
                    if gi < ng:
                        emit_scores_exp(gi)
                    for ci in ctx_by_slot.get(gi, []):
                        emit_ctx(ci)
                    # staggered boundary work per finished block
                    for qb in range(NQB):
                        base = last_emit[qb]
                        if gi == base + EV_OFF:
                            emit_evict_recip(qb)
                        elif gi == base + NORM_OFF:
                            emit_norm(qb, [0, 1])
                        elif gi == base + NORM_OFF + 1:
                            emit_norm(qb, [2, 3])
                            emit_outproj(qb, [0])
                        elif gi == base + PROJ_OFF:
                            emit_outproj(qb, [1, 2, 3], final=True)
                scp_cm.__exit__(None, None, None)
    if legalize:
        _legalize_sync_waits(nc)
    return nc


_NC_CACHE = None


def _get_nc():
    global _NC_CACHE
    if _NC_CACHE is None:
        _NC_CACHE = _build_nc()
    return _NC_CACHE


def _ks_layout(a, nk, cols):
    """[nk*128, cols] -> [128, nk*cols] with [p, k*cols+c] = a[k*128+p, c]."""
    return np.ascontiguousarray(
        a.reshape(nk, 128, cols).transpose(1, 0, 2).reshape(128, nk * cols)
    )


def _prep_in_maps(x, w_qkv, b_qkv, w_out, b_out):
    x = np.asarray(x, dtype=np.float32)
    w_qkv = np.asarray(w_qkv, dtype=np.float32)
    b_qkv = np.asarray(b_qkv, dtype=np.float32)
    w_out = np.asarray(w_out, dtype=np.float32)
    b_out = np.asarray(b_out, dtype=np.float32)

    wqk_l = _ks_layout(w_qkv[:, : 2 * H], 2, 512).astype(NPBF16)

    # q/k bias as per-partition vectors, one column per phase-1 tile
    bqk_t = np.ascontiguousarray(
        b_qkv[: 2 * H].reshape(4, 128).T).astype(np.float32)

    # v weights: head-pair slots of 66: [v_2p |1| v_2p+1 |1] (ones via bias row)
    wpad = np.zeros((H, 264), np.float32)
    bvr = np.zeros((1, 264), np.float32)
    for h in range(NH):
        c0 = (h // 2) * 66 + (h % 2) * 33
        wpad[:, c0: c0 + 32] = w_qkv[:, 2 * H + h * HD: 2 * H + (h + 1) * HD]
        bvr[0, c0: c0 + 32] = b_qkv[2 * H + h * HD: 2 * H + (h + 1) * HD]
        bvr[0, c0 + 32] = 1.0
    wv_l = _ks_layout(wpad, 2, 264).astype(NPBF16)

    # w_out rows permuted into ctxT slot layout: per pair-block of 128 rows:
    # [head 2p (32) | b_out row (pair 0 only) | 31 zeros | head 2p+1 (32) | 32 zeros]
    wo_perm = np.zeros((512, H), np.float32)
    for pair in range(4):
        r0 = pair * 128
        wo_perm[r0: r0 + 32, :] = w_out[(2 * pair) * HD: (2 * pair + 1) * HD, :]
        wo_perm[r0 + 64: r0 + 96, :] = w_out[(2 * pair + 1) * HD: (2 * pair + 2) * HD, :]
    wo_perm[32, :] = b_out  # ctxT row 32 of pair 0 is rs*1/rs = 1
    wo_l = _ks_layout(wo_perm, 4, 256).astype(NPBF16)

    shared = {
        "wqk": wqk_l,
        "wv": wv_l,
        "bqk": bqk_t,
        "bv": bvr.astype(NPBF16),
        "wo": wo_l,
        "ones": np.ones((1, 512), NPBF16),
    }
    in_maps = []
    for b in range(B):
        xtm = _ks_layout(np.ascontiguousarray(x[b].T), 2, S).astype(NPBF16)
        in_maps.append({"xt": xtm, **shared})
    return in_maps


def kernel(x, w_qkv, b_qkv, w_out, b_out):
    in_maps = _prep_in_maps(x, w_qkv, b_qkv, w_out, b_out)
    nc = _get_nc()
    res = run_bass_kernel_spmd(nc, in_maps, list(range(N_CORES)), **TRACE_OPTS)
    global LAST_RESULT
    LAST_RESULT = res
    return np.stack([res.results[b]["out"] for b in range(B)], axis=0)


# revision 22
# speedup vs baseline: 1.5037x; 1.0137x over previous
"""Multi-head self-attention (B=8, S=2048, H=256, NH=8, HD=32) on 8 TRN2 cores.

v4: data-parallel over batch (1 batch element/core). Lessons baked in:
  - PE streams at ~0.42ns/col when matmuls are >=512 cols; 256-col
    matmuls can't hide LDWEIGHTS (~100-150ns) -> keep 512-col streams
    (4 q-blocks of 512, v2's shapes).
  - matmuls at different PE ROW tile positions must never write the
    same PSUM bank (hw fault): scores use 4 position-locked banks.
  - flat global group stream; ctx trails scores by CTX_DELAY groups so
    block-boundary work hides behind queued PE work; boundary emissions
    staggered (evict+recip / normalize / out-proj) to avoid in-order
    engine stalls.
  - boundary chain in bf16: stg evict bf16 (rowsums ride along), bf16
    recip roundtrip, bf16 broadcast, 2x-mode DVE normalize.
  - out-projection as single 128-wide stationary tiles (16384 cols vs
    v2's 32768); q/k bias folded into phase-1 evictions as per-partition
    vectors (drops 16 rank-1 matmuls).
"""
import numpy as np
import ml_dtypes

import bass_rust
import concourse.bass as bass
import concourse.mybir as mybir
import concourse.tile as tile
from concourse.bass_utils import run_bass_kernel_spmd

BF16 = mybir.dt.bfloat16
F32 = mybir.dt.float32
I16 = mybir.dt.int16
NPBF16 = ml_dtypes.bfloat16

B, S, H = 8, 2048, 256
NH, HD = 8, 32
SCALE = 1.0 / float(np.sqrt(HD))
N_CORES = 8

LOG2E = 1.4426950408889634
# DVE bit-hack exp constants (c centers the mantissa-interp sawtooth; the
# +0.5 assumes truncation on float->int convert)
HACK_C = 5.5
HACK_ROUND = True
A_HACK = float(LOG2E * 128.0 * SCALE)
B_HACK = float(127.0 * 128.0 - HACK_C + (0.0 if HACK_ROUND else 0.5))

NQB = 4          # q-blocks
QBS = 512        # q-block size
CTX_DELAY = 6    # groups between a group's scores and its ctx matmuls
EV_OFF = 1       # boundary stagger (slots after a block's last ctx batch)
NORM_OFF = 4
PROJ_OFF = 6
ACT_SHARE = 8    # of every 16 exp units, this many go to ScalarE
MAX_GI = None    # truncate slot loop (bisect)

TRACE_OPTS = {}
LAST_RESULT = None


def _legalize_sync_waits(nc):
    """Split multi-wait sync_infos onto NoOp carriers (walrus allows 1/inst)."""
    n = 0
    for f in nc.m.functions:
        for bb in f.blocks:
            insts = bb.instructions
            i = 0
            while i < len(insts):
                inst = insts[i]
                si = inst.sync_info
                if si is not None and len(si.on_wait) > 1:
                    waits = list(si.on_wait)
                    carriers = []
                    for w in waits[:-1]:
                        carriers.append(
                            mybir.InstNoOp(
                                name=f"{inst.name}-w{n}",
                                sync_info=mybir.SyncInfo(on_wait=[w], on_update=[]),
                                bass_nofuse=True,
                                engine=inst.engine,
                            )
                        )
                        n += 1
                    inst.sync_info = bass_rust.SyncInfo(
                        on_wait=waits[-1:], on_update=list(si.on_update)
                    )
                    insts[i:i] = carriers
                    i += len(carriers)
                i += 1
    return n


def _build_nc(legalize=True):
    nc = bass.Bass()
    xt = nc.dram_tensor("xt", [128, 2 * S], BF16, kind="ExternalInput")
    wqk = nc.dram_tensor("wqk", [128, 2 * 512], BF16, kind="ExternalInput")
    bqk = nc.dram_tensor("bqk", [128, 4], F32, kind="ExternalInput")
    bv = nc.dram_tensor("bv", [1, 264], BF16, kind="ExternalInput")
    wv = nc.dram_tensor("wv", [128, 2 * 264], BF16, kind="ExternalInput")
    wo = nc.dram_tensor("wo", [128, 4 * 256], BF16, kind="ExternalInput")
    ones = nc.dram_tensor("ones", [1, 512], BF16, kind="ExternalInput")
    out = nc.dram_tensor("out", [S, H], F32, kind="ExternalOutput")
    # rowsum gather / reciprocal round-trip scratch ([2,2048] <-> [128,32])
    rscr = nc.dram_tensor("rscr", [2, 2048], BF16)
    rscr2 = nc.dram_tensor("rscr2", [2, 2048], BF16)

    EXP = mybir.ActivationFunctionType.Exp
    IDENT = mybir.ActivationFunctionType.Identity

    with tile.TileContext(nc) as tc:
        with (
            tc.tile_pool(name="const", bufs=1) as const,
            tc.tile_pool(name="etp", bufs=15) as etp,
            tc.tile_pool(name="ctp", bufs=2) as ctp,
            tc.tile_pool(name="stp", bufs=2) as stp,
            tc.tile_pool(name="osb", bufs=2) as osb,
        ):
            # ---- input DMAs, in first-use order ----
            wqk_sb = const.tile([128, 2 * 512], BF16, tag="wqk")
            nc.sync.dma_start(out=wqk_sb, in_=wqk[:, :])
            xt_sb = const.tile([128, 2 * S], BF16, tag="xt")
            for ch in (0, 2, 1, 3):  # phase 1 consumes ks-paired halves
                nc.sync.dma_start(
                    out=xt_sb[:, ch * 1024: ch * 1024 + 1024],
                    in_=xt[:, ch * 1024: ch * 1024 + 1024])
            bqk_sb = const.tile([128, 4], F32, tag="bqk")
            nc.sync.dma_start(out=bqk_sb, in_=bqk[:, :])
            wv_sb = const.tile([128, 2 * 264], BF16, tag="wv")
            nc.sync.dma_start(out=wv_sb, in_=wv[:, :])
            bv_sb = const.tile([1, 264], BF16, tag="bv")
            nc.sync.dma_start(out=bv_sb, in_=bv[:, :])
            ones_sb = const.tile([1, 512], BF16, tag="ones")
            nc.sync.dma_start(out=ones_sb, in_=ones[:, :])
            wo_sb = const.tile([128, 4 * 256], BF16, tag="wo")
            nc.sync.dma_start(out=wo_sb, in_=wo[:, :])

            qT_sb = const.tile([128, 2 * S], BF16, tag="qT")
            kT_sb = const.tile([128, 2 * S], BF16, tag="kT")
            v_sb = const.tile([128, 16 * 264], BF16, tag="v")

            # persistent PSUM: ctx accumulators (4 banks, one per pair)
            with (
                tc.tile_pool(name="cxp", bufs=1, space="PSUM") as cxp,
            ):
                ctx_ps = cxp.tile([128, 2048], F32, tag="ctx", name="ctx")

                # ---- warmup (p-state ramp) while input DMAs land ----
                warm_sb = const.tile([128, 512], BF16, tag="warm")
                nc.gpsimd.memset(warm_sb, 0.0)
                for r in range(12):
                    nc.tensor.matmul(
                        out=ctx_ps[:, 0:512], lhsT=warm_sb[:, 0:128],
                        rhs=warm_sb[:, :], start=True, stop=True,
                    )
                # zero the never-matmul-written ctx rows so full-height
                # eviction reads finite values (persist across q-blocks)
                nc.vector.memset(ctx_ps[32:64, :], 0.0)
                nc.vector.memset(ctx_ps[96:128, :], 0.0)

                # ---- phase 1: qT/kT [feature, s]; bias folded into the
                #      evictions as per-partition vectors ----
                p12 = tc.tile_pool(name="p12", bufs=4, space="PSUM")
                scp12 = p12.__enter__()
                for nb in range(4):  # s blocks of 512
                    for t in range(4):  # feature tiles: q0,q1,k0,k1
                        ps = scp12.tile([128, 512], F32, tag="sc",
                                        name=f"p1_{t}_{nb}")
                        for ks in range(2):
                            nc.tensor.matmul(
                                out=ps,
                                lhsT=wqk_sb[:, ks * 512 + t * 128: ks * 512 + t * 128 + 128],
                                rhs=xt_sb[:, ks * S + nb * 512: ks * S + nb * 512 + 512],
                                start=(ks == 0), stop=(ks == 1),
                            )
                        dst = (qT_sb if t < 2 else kT_sb)[
                            :, (t % 2) * S + nb * 512: (t % 2) * S + nb * 512 + 512
                        ]
                        if t % 2 == 0:
                            nc.scalar.activation(
                                out=dst, in_=ps, func=IDENT,
                                bias=bqk_sb[:, t: t + 1], scale=1.0,
                            )
                        else:
                            nc.vector.tensor_scalar(
                                out=dst, in0=ps,
                                scalar1=bqk_sb[:, t: t + 1], scalar2=None,
                                op0=mybir.AluOpType.add,
                            )

                # ---- phase 2: v (padded 66-wide head-pair slots, ones col
                #      per head for rowsums; bias row plants the ones) ----
                for st in range(16):
                    ps = scp12.tile([128, 512], F32, tag="sc", name=f"p2_{st}")
                    for ks in range(2):
                        nc.tensor.matmul(
                            out=ps[:, 0:264],
                            lhsT=xt_sb[:, ks * S + st * 128: ks * S + st * 128 + 128],
                            rhs=wv_sb[:, ks * 264: ks * 264 + 264],
                            start=(ks == 0), stop=False,
                        )
                    nc.tensor.matmul(
                        out=ps[:, 0:264],
                        lhsT=ones_sb[0:1, 0:128],
                        rhs=bv_sb[0:1, 0:264],
                        start=False, stop=True,
                    )
                    dst = v_sb[:, st * 264: st * 264 + 264]
                    if st % 2 == 0:
                        nc.scalar.copy(out=dst, in_=ps[:, 0:264])
                    else:
                        nc.vector.tensor_copy(out=dst, in_=ps[:, 0:264])

                p12.__exit__(None, None, None)

                # ---- phase 3: attention, flat stream of (qb, g, kt) groups ----
                scp_cm = tc.tile_pool(name="scp", bufs=1, space="PSUM")
                scp = scp_cm.__enter__()
                ng = NQB * 32

                eT_ring = {}      # gi -> eT tile awaiting ctx
                blk = {}          # qb -> dict of boundary tiles

                # one persistent [128,512] bank per PE row tile position:
                # matmuls at different ROW tile positions must not write the
                # same PSUM bank (hw fault).
                sc_banks = [
                    scp.tile([128, 512], F32, tag=f"sc{i}", name=f"sc{i}")
                    for i in range(4)
                ]

                def emit_scores_exp(gi):
                    qb, r = divmod(gi, 32)
                    kt, g = divmod(r, 2)
                    eT = etp.tile([128, 2048], BF16, tag="eT", name=f"eT{gi}")
                    for i in range(4):
                        nc.tensor.matmul(
                            out=sc_banks[i],
                            lhsT=kT_sb[32 * i: 32 * i + 32,
                                       g * S + kt * 128: g * S + kt * 128 + 128],
                            rhs=qT_sb[32 * i: 32 * i + 32,
                                      g * S + qb * QBS: g * S + qb * QBS + QBS],
                            start=True, stop=True,
                            tile_position=(32 * i, 0),
                        )
                        eT_h = eT[:, i * 512: i * 512 + 512]
                        if ((gi * 4 + i) * ACT_SHARE) % 16 < ACT_SHARE:
                            nc.scalar.activation(
                                out=eT_h, in_=sc_banks[i], func=EXP,
                                scale=SCALE,
                            )
                        else:
                            nc.vector.tensor_scalar(
                                out=eT_h.bitcast(I16), in0=sc_banks[i],
                                scalar1=A_HACK, scalar2=B_HACK,
                                op0=mybir.AluOpType.mult,
                                op1=mybir.AluOpType.add,
                            )
                    eT_ring[gi] = eT

                def emit_ctx(ci):
                    qb, r = divmod(ci, 32)
                    kt, g = divmod(r, 2)
                    eT = eT_ring.pop(ci)
                    for pi in range(2):
                        pair = g * 2 + pi
                        vc = kt * 264 + pair * 66
                        nc.tensor.matmul(
                            out=ctx_ps[0:33, pair * 512: pair * 512 + 512],
                            lhsT=v_sb[:, vc: vc + 33],
                            rhs=eT[:, (2 * pi) * 512: (2 * pi) * 512 + 512],
                            start=(kt == 0), stop=(kt == 15),
                            tile_position=(0, 0), skip_group_check=True,
                        )
                        nc.tensor.matmul(
                            out=ctx_ps[64:97, pair * 512: pair * 512 + 512],
                            lhsT=v_sb[:, vc + 33: vc + 66],
                            rhs=eT[:, (2 * pi + 1) * 512: (2 * pi + 1) * 512 + 512],
                            start=(kt == 0), stop=(kt == 15),
                            tile_position=(0, 64), skip_group_check=True,
                        )

                def emit_evict_recip(qb):
                    stg = stp.tile([128, 2048], BF16, tag="stg", name=f"stg{qb}")
                    nc.scalar.copy(out=stg, in_=ctx_ps)
                    # rowsums (rows 32/96) -> [128,32] transpose via direct
                    # SBUF->SBUF DMA -> recip -> DRAM rows -> per-half bcast
                    rsg = osb.tile([128, 32], BF16, tag="rsg", name=f"rsg{qb}")
                    nc.sync.dma_start(out=rsg[0:64, :], in_=stg[32:33, :])
                    nc.sync.dma_start(out=rsg[64:128, :], in_=stg[96:97, :])
                    with nc.allow_low_precision(
                        reason="softmax rowsum recip in bf16; ~0.4% rel"
                    ):
                        nc.vector.reciprocal(out=rsg, in_=rsg)
                    nc.sync.dma_start(out=rscr2[:, :], in_=rsg)
                    rcb = stp.tile([128, 2048], BF16, tag="rcb", name=f"rcb{qb}")
                    nc.sync.dma_start(
                        out=rcb[0:64, :],
                        in_=rscr2[0:1, :].to_broadcast((64, 2048)),
                    )
                    nc.sync.dma_start(
                        out=rcb[64:128, :],
                        in_=rscr2[1:2, :].to_broadcast((64, 2048)),
                    )
                    blk[qb] = {"stg": stg, "rcb": rcb}

                def emit_norm(qb, sts):
                    b = blk[qb]
                    if "ctxT" not in b:
                        b["ctxT"] = ctp.tile([128, 2048], BF16, tag="ctxT",
                                             name=f"ctxT{qb}")
                    def chunk(t, st):
                        # cols {pair*512 + st*128 .. +128, pair=0..4}
                        return t.rearrange("p (a b) -> p a b", a=4)[
                            :, :, st * 128: st * 128 + 128]
                    for st in sts:
                        nc.vector.tensor_mul(
                            out=chunk(b["ctxT"][:, :], st),
                            in0=chunk(b["stg"][:, :], st),
                            in1=chunk(b["rcb"][:, :], st),
                        )

                def emit_outproj(qb, sts, final=False):
                    b = blk[qb]
                    ctxT = b["ctxT"]
                    if "po" not in b:
                        b["po"] = osb.tile([128, 1024], F32, tag="ot",
                                           name=f"ot{qb}")
                    po = b["po"]
                    if final:
                        blk.pop(qb)
                    for st in sts:
                        for pair in range(4):
                            nc.tensor.matmul(
                                out=ctx_ps[:, st * 512: st * 512 + 256],
                                lhsT=ctxT[:, pair * 512 + st * 128:
                                          pair * 512 + st * 128 + 128],
                                rhs=wo_sb[:, pair * 256: pair * 256 + 256],
                                start=(pair == 0), stop=(pair == 3),
                                tile_position=(0, 0),
                                skip_group_check=True,
                            )
                        nc.scalar.copy(
                            out=po[:, st * 256: st * 256 + 256],
                            in_=ctx_ps[:, st * 512: st * 512 + 256],
                        )
                        nc.sync.dma_start(
                            out=out[qb * QBS + st * 128: qb * QBS + st * 128 + 128, :],
                            in_=po[:, st * 256: st * 256 + 256],
                        )

                # ctx schedule: each block's first ctx batches are delayed
                # by D extra slots (scores keep the PE busy meanwhile) so the
                # PREVIOUS block's out-proj can finish with the ctx banks
                # free; the backlog catches up at 2 ctx batches per slot.
                D = 7

                def ctx_slot(ci):
                    qb, l = divmod(ci, 32)
                    if l < 2 * D:
                        return qb * 32 + CTX_DELAY + D + (l + 1) // 2
                    return ci + CTX_DELAY

                ctx_by_slot = {}
                for ci in range(ng):
                    ctx_by_slot.setdefault(ctx_slot(ci), []).append(ci)
                last_emit = {qb: ctx_slot(qb * 32 + 31) for qb in range(NQB)}
                n_slots = (max(last_emit.values()) + PROJ_OFF + 1) if ng else 0
                if MAX_GI is not None:
                    n_slots = min(n_slots, MAX_GI)
                for gi in range(n_slots):
                    if gi < ng:
                        emit_scores_exp(gi)
                    for ci in ctx_by_slot.get(gi, []):
                        emit_ctx(ci)
                    # staggered boundary work per finished block
                    for qb in range(NQB):
                        base = last_emit[qb]
                        if gi == base + EV_OFF:
                            emit_evict_recip(qb)
                        elif gi == base + NORM_OFF:
                            emit_norm(qb, [0, 1])
                        elif gi == base + NORM_OFF + 1:
                            emit_norm(qb, [2, 3])
                            emit_outproj(qb, [0])
                        elif gi == base + PROJ_OFF:
                            emit_outproj(qb, [1, 2, 3], final=True)
                scp_cm.__exit__(None, None, None)
    if legalize:
        _legalize_sync_waits(nc)
    return nc


_NC_CACHE = None


def _get_nc():
    global _NC_CACHE
    if _NC_CACHE is None:
        _NC_CACHE = _build_nc()
    return _NC_CACHE


def _ks_layout(a, nk, cols):
    """[nk*128, cols] -> [128, nk*cols] with [p, k*cols+c] = a[k*128+p, c]."""
    return np.ascontiguousarray(
        a.reshape(nk, 128, cols).transpose(1, 0, 2).reshape(128, nk * cols)
    )


def _prep_in_maps(x, w_qkv, b_qkv, w_out, b_out):
    x = np.asarray(x, dtype=np.float32)
    w_qkv = np.asarray(w_qkv, dtype=np.float32)
    b_qkv = np.asarray(b_qkv, dtype=np.float32)
    w_out = np.asarray(w_out, dtype=np.float32)
    b_out = np.asarray(b_out, dtype=np.float32)

    wqk_l = _ks_layout(w_qkv[:, : 2 * H], 2, 512).astype(NPBF16)

    # q/k bias as per-partition vectors, one column per phase-1 tile
    bqk_t = np.ascontiguousarray(
        b_qkv[: 2 * H].reshape(4, 128).T).astype(np.float32)

    # v weights: head-pair slots of 66: [v_2p |1| v_2p+1 |1] (ones via bias row)
    wpad = np.zeros((H, 264), np.float32)
    bvr = np.zeros((1, 264), np.float32)
    for h in range(NH):
        c0 = (h // 2) * 66 + (h % 2) * 33
        wpad[:, c0: c0 + 32] = w_qkv[:, 2 * H + h * HD: 2 * H + (h + 1) * HD]
        bvr[0, c0: c0 + 32] = b_qkv[2 * H + h * HD: 2 * H + (h + 1) * HD]
        bvr[0, c0 + 32] = 1.0
    wv_l = _ks_layout(wpad, 2, 264).astype(NPBF16)

    # w_out rows permuted into ctxT slot layout: per pair-block of 128 rows:
    # [head 2p (32) | b_out row (pair 0 only) | 31 zeros | head 2p+1 (32) | 32 zeros]
    wo_perm = np.zeros((512, H), np.float32)
    for pair in range(4):
        r0 = pair * 128
        wo_perm[r0: r0 + 32, :] = w_out[(2 * pair) * HD: (2 * pair + 1) * HD, :]
        wo_perm[r0 + 64: r0 + 96, :] = w_out[(2 * pair + 1) * HD: (2 * pair + 2) * HD, :]
    wo_perm[32, :] = b_out  # ctxT row 32 of pair 0 is rs*1/rs = 1
    wo_l = _ks_layout(wo_perm, 4, 256).astype(NPBF16)

    shared = {
        "wqk": wqk_l,
        "wv": wv_l,
        "bqk": bqk_t,
        "bv": bvr.astype(NPBF16),
        "wo": wo_l,
        "ones": np.ones((1, 512), NPBF16),
    }
    in_maps = []
    for b in range(B):
        xtm = _ks_layout(np.ascontiguousarray(x[b].T), 2, S).astype(NPBF16)
        in_maps.append({"xt": xtm, **shared})
    return in_maps


def kernel(x, w_qkv, b_qkv, w_out, b_out):
    in_maps = _prep_in_maps(x, w_qkv, b_qkv, w_out, b_out)
    nc = _get_nc()
    res = run_bass_kernel_spmd(nc, in_maps, list(range(N_CORES)), **TRACE_OPTS)
    global LAST_RESULT
    LAST_RESULT = res
    return np.stack([res.results[b]["out"] for b in range(B)], axis=0)


# revision 24
# speedup vs baseline: 1.5199x; 1.0107x over previous
"""Multi-head self-attention (B=8, S=2048, H=256, NH=8, HD=32) on 8 TRN2 cores.

v4: data-parallel over batch (1 batch element/core). Lessons baked in:
  - PE streams at ~0.42ns/col when matmuls are >=512 cols; 256-col
    matmuls can't hide LDWEIGHTS (~100-150ns) -> keep 512-col streams
    (4 q-blocks of 512, v2's shapes).
  - matmuls at different PE ROW tile positions must never write the
    same PSUM bank (hw fault): scores use 4 position-locked banks.
  - flat global group stream; ctx trails scores by CTX_DELAY groups so
    block-boundary work hides behind queued PE work; boundary emissions
    staggered (evict+recip / normalize / out-proj) to avoid in-order
    engine stalls.
  - boundary chain in bf16: stg evict bf16 (rowsums ride along), bf16
    recip roundtrip, bf16 broadcast, 2x-mode DVE normalize.
  - out-projection as single 128-wide stationary tiles (16384 cols vs
    v2's 32768); q/k bias folded into phase-1 evictions as per-partition
    vectors (drops 16 rank-1 matmuls).
"""
import numpy as np
import ml_dtypes

import bass_rust
import concourse.bass as bass
import concourse.mybir as mybir
import concourse.tile as tile
from concourse.bass_utils import run_bass_kernel_spmd

BF16 = mybir.dt.bfloat16
F32 = mybir.dt.float32
I16 = mybir.dt.int16
NPBF16 = ml_dtypes.bfloat16

B, S, H = 8, 2048, 256
NH, HD = 8, 32
SCALE = 1.0 / float(np.sqrt(HD))
N_CORES = 8

LOG2E = 1.4426950408889634
# DVE bit-hack exp constants (c centers the mantissa-interp sawtooth; the
# +0.5 assumes truncation on float->int convert)
HACK_C = 5.5
HACK_ROUND = True
A_HACK = float(LOG2E * 128.0 * SCALE)
B_HACK = float(127.0 * 128.0 - HACK_C + (0.0 if HACK_ROUND else 0.5))

NQB = 4          # q-blocks
QBS = 512        # q-block size
CTX_DELAY = 6    # groups between a group's scores and its ctx matmuls
EV_OFF = 1       # boundary stagger (slots after a block's last ctx batch)
NORM_OFF = 4
PROJ_OFF = 6
ACT_SHARE = 8    # of every 16 exp units, this many go to ScalarE
MAX_GI = None    # truncate slot loop (bisect)

TRACE_OPTS = {}
LAST_RESULT = None


def _legalize_sync_waits(nc):
    """Split multi-wait sync_infos onto NoOp carriers (walrus allows 1/inst)."""
    n = 0
    for f in nc.m.functions:
        for bb in f.blocks:
            insts = bb.instructions
            i = 0
            while i < len(insts):
                inst = insts[i]
                si = inst.sync_info
                if si is not None and len(si.on_wait) > 1:
                    waits = list(si.on_wait)
                    carriers = []
                    for w in waits[:-1]:
                        carriers.append(
                            mybir.InstNoOp(
                                name=f"{inst.name}-w{n}",
                                sync_info=mybir.SyncInfo(on_wait=[w], on_update=[]),
                                bass_nofuse=True,
                                engine=inst.engine,
                            )
                        )
                        n += 1
                    inst.sync_info = bass_rust.SyncInfo(
                        on_wait=waits[-1:], on_update=list(si.on_update)
                    )
                    insts[i:i] = carriers
                    i += len(carriers)
                i += 1
    return n


def _build_nc(legalize=True):
    nc = bass.Bass()
    xt = nc.dram_tensor("xt", [128, 2 * S], BF16, kind="ExternalInput")
    wqk = nc.dram_tensor("wqk", [128, 2 * 512], BF16, kind="ExternalInput")
    bqk = nc.dram_tensor("bqk", [128, 4], F32, kind="ExternalInput")
    bv = nc.dram_tensor("bv", [1, 264], BF16, kind="ExternalInput")
    wv = nc.dram_tensor("wv", [128, 2 * 264], BF16, kind="ExternalInput")
    wo = nc.dram_tensor("wo", [128, 4 * 256], BF16, kind="ExternalInput")
    ones = nc.dram_tensor("ones", [1, 512], BF16, kind="ExternalInput")
    out = nc.dram_tensor("out", [S, H], F32, kind="ExternalOutput")
    # rowsum gather / reciprocal round-trip scratch ([2,2048] <-> [128,32])
    rscr = nc.dram_tensor("rscr", [2, 2048], BF16)
    rscr2 = nc.dram_tensor("rscr2", [2, 2048], BF16)

    EXP = mybir.ActivationFunctionType.Exp
    IDENT = mybir.ActivationFunctionType.Identity

    with tile.TileContext(nc) as tc:
        with (
            tc.tile_pool(name="const", bufs=1) as const,
            tc.tile_pool(name="etp", bufs=15) as etp,
            tc.tile_pool(name="ctp", bufs=2) as ctp,
            tc.tile_pool(name="stp", bufs=2) as stp,
            tc.tile_pool(name="osb", bufs=2) as osb,
        ):
            # ---- input DMAs, in first-use order ----
            wqk_sb = const.tile([128, 2 * 512], BF16, tag="wqk")
            nc.sync.dma_start(out=wqk_sb, in_=wqk[:, :])
            xt_sb = const.tile([128, 2 * S], BF16, tag="xt")
            for ch in (0, 2, 1, 3):  # phase 1 consumes ks-paired halves
                nc.sync.dma_start(
                    out=xt_sb[:, ch * 1024: ch * 1024 + 1024],
                    in_=xt[:, ch * 1024: ch * 1024 + 1024])
            bqk_sb = const.tile([128, 4], F32, tag="bqk")
            nc.sync.dma_start(out=bqk_sb, in_=bqk[:, :])
            wv_sb = const.tile([128, 2 * 264], BF16, tag="wv")
            nc.sync.dma_start(out=wv_sb, in_=wv[:, :])
            bv_sb = const.tile([1, 264], BF16, tag="bv")
            nc.sync.dma_start(out=bv_sb, in_=bv[:, :])
            ones_sb = const.tile([1, 512], BF16, tag="ones")
            nc.sync.dma_start(out=ones_sb, in_=ones[:, :])
            wo_sb = const.tile([128, 4 * 256], BF16, tag="wo")
            nc.sync.dma_start(out=wo_sb, in_=wo[:, :])

            qT_sb = const.tile([128, 2 * S], BF16, tag="qT")
            kT_sb = const.tile([128, 2 * S], BF16, tag="kT")
            v_sb = const.tile([128, 16 * 264], BF16, tag="v")

            # persistent PSUM: ctx accumulators (4 banks, one per pair)
            with (
                tc.tile_pool(name="cxp", bufs=1, space="PSUM") as cxp,
            ):
                ctx_ps = cxp.tile([128, 2048], F32, tag="ctx", name="ctx")

                # ---- warmup (p-state ramp) while input DMAs land ----
                warm_sb = const.tile([128, 512], BF16, tag="warm")
                nc.gpsimd.memset(warm_sb, 0.0)
                for r in range(12):
                    nc.tensor.matmul(
                        out=ctx_ps[:, 0:512], lhsT=warm_sb[:, 0:128],
                        rhs=warm_sb[:, :], start=True, stop=True,
                    )
                # zero the never-matmul-written ctx rows so full-height
                # eviction reads finite values (persist across q-blocks)
                nc.vector.memset(ctx_ps[32:64, :], 0.0)
                nc.vector.memset(ctx_ps[96:128, :], 0.0)

                # ---- phase 1: qT/kT [feature, s]; bias folded into the
                #      evictions as per-partition vectors ----
                p12 = tc.tile_pool(name="p12", bufs=4, space="PSUM")
                scp12 = p12.__enter__()
                for nb in range(4):  # s blocks of 512
                    for t in range(4):  # feature tiles: q0,q1,k0,k1
                        ps = scp12.tile([128, 512], F32, tag="sc",
                                        name=f"p1_{t}_{nb}")
                        for ks in range(2):
                            nc.tensor.matmul(
                                out=ps,
                                lhsT=wqk_sb[:, ks * 512 + t * 128: ks * 512 + t * 128 + 128],
                                rhs=xt_sb[:, ks * S + nb * 512: ks * S + nb * 512 + 512],
                                start=(ks == 0), stop=(ks == 1),
                            )
                        dst = (qT_sb if t < 2 else kT_sb)[
                            :, (t % 2) * S + nb * 512: (t % 2) * S + nb * 512 + 512
                        ]
                        if t % 2 == 0:
                            nc.scalar.activation(
                                out=dst, in_=ps, func=IDENT,
                                bias=bqk_sb[:, t: t + 1], scale=1.0,
                            )
                        else:
                            nc.vector.tensor_scalar(
                                out=dst, in0=ps,
                                scalar1=bqk_sb[:, t: t + 1], scalar2=None,
                                op0=mybir.AluOpType.add,
                            )

                # ---- phase 2: v (padded 66-wide head-pair slots, ones col
                #      per head for rowsums; bias row plants the ones) ----
                for st in range(16):
                    ps = scp12.tile([128, 512], F32, tag="sc", name=f"p2_{st}")
                    for ks in range(2):
                        nc.tensor.matmul(
                            out=ps[:, 0:264],
                            lhsT=xt_sb[:, ks * S + st * 128: ks * S + st * 128 + 128],
                            rhs=wv_sb[:, ks * 264: ks * 264 + 264],
                            start=(ks == 0), stop=False,
                        )
                    nc.tensor.matmul(
                        out=ps[:, 0:264],
                        lhsT=ones_sb[0:1, 0:128],
                        rhs=bv_sb[0:1, 0:264],
                        start=False, stop=True,
                    )
                    dst = v_sb[:, st * 264: st * 264 + 264]
                    if st % 2 == 0:
                        nc.scalar.copy(out=dst, in_=ps[:, 0:264])
                    else:
                        nc.vector.tensor_copy(out=dst, in_=ps[:, 0:264])

                p12.__exit__(None, None, None)

                # ---- phase 3: attention, flat stream of (qb, g, kt) groups ----
                scp_cm = tc.tile_pool(name="scp", bufs=1, space="PSUM")
                scp = scp_cm.__enter__()
                ng = NQB * 32

                eT_ring = {}      # gi -> eT tile awaiting ctx
                blk = {}          # qb -> dict of boundary tiles

                # one persistent [128,512] bank per PE row tile position:
                # matmuls at different ROW tile positions must not write the
                # same PSUM bank (hw fault).
                sc_banks = [
                    scp.tile([128, 512], F32, tag=f"sc{i}", name=f"sc{i}")
                    for i in range(4)
                ]

                def emit_scores_exp(gi):
                    qb, r = divmod(gi, 32)
                    kt, g = divmod(r, 2)
                    eT = etp.tile([128, 2048], BF16, tag="eT", name=f"eT{gi}")
                    for i in range(4):
                        nc.tensor.matmul(
                            out=sc_banks[i],
                            lhsT=kT_sb[32 * i: 32 * i + 32,
                                       g * S + kt * 128: g * S + kt * 128 + 128],
                            rhs=qT_sb[32 * i: 32 * i + 32,
                                      g * S + qb * QBS: g * S + qb * QBS + QBS],
                            start=True, stop=True,
                            tile_position=(32 * i, 0),
                        )
                        eT_h = eT[:, i * 512: i * 512 + 512]
                        if ((gi * 4 + i) * ACT_SHARE) % 16 < ACT_SHARE:
                            nc.scalar.activation(
                                out=eT_h, in_=sc_banks[i], func=EXP,
                                scale=SCALE,
                            )
                        else:
                            nc.vector.tensor_scalar(
                                out=eT_h.bitcast(I16), in0=sc_banks[i],
                                scalar1=A_HACK, scalar2=B_HACK,
                                op0=mybir.AluOpType.mult,
                                op1=mybir.AluOpType.add,
                            )
                    eT_ring[gi] = eT

                def emit_ctx(ci):
                    qb, r = divmod(ci, 32)
                    kt, g = divmod(r, 2)
                    eT = eT_ring.pop(ci)
                    for pi in range(2):
                        pair = g * 2 + pi
                        vc = kt * 264 + pair * 66
                        nc.tensor.matmul(
                            out=ctx_ps[0:33, pair * 512: pair * 512 + 512],
                            lhsT=v_sb[:, vc: vc + 33],
                            rhs=eT[:, (2 * pi) * 512: (2 * pi) * 512 + 512],
                            start=(kt == 0), stop=(kt == 15),
                            tile_position=(0, 0), skip_group_check=True,
                        )
                        nc.tensor.matmul(
                            out=ctx_ps[64:97, pair * 512: pair * 512 + 512],
                            lhsT=v_sb[:, vc + 33: vc + 66],
                            rhs=eT[:, (2 * pi + 1) * 512: (2 * pi + 1) * 512 + 512],
                            start=(kt == 0), stop=(kt == 15),
                            tile_position=(0, 64), skip_group_check=True,
                        )

                def emit_evict_recip(qb):
                    stg = stp.tile([128, 2048], BF16, tag="stg", name=f"stg{qb}")
                    nc.scalar.copy(out=stg, in_=ctx_ps)
                    # rowsums (rows 32/96) -> [128,32] transpose via direct
                    # SBUF->SBUF DMA -> recip -> DRAM rows -> per-half bcast
                    rsg = osb.tile([128, 32], BF16, tag="rsg", name=f"rsg{qb}")
                    nc.sync.dma_start(out=rsg[0:64, :], in_=stg[32:33, :])
                    nc.sync.dma_start(out=rsg[64:128, :], in_=stg[96:97, :])
                    with nc.allow_low_precision(
                        reason="softmax rowsum recip in bf16; ~0.4% rel"
                    ):
                        nc.vector.reciprocal(out=rsg, in_=rsg)
                    nc.sync.dma_start(out=rscr2[:, :], in_=rsg)
                    rcb = stp.tile([128, 2048], BF16, tag="rcb", name=f"rcb{qb}")
                    nc.sync.dma_start(
                        out=rcb[0:64, :],
                        in_=rscr2[0:1, :].to_broadcast((64, 2048)),
                    )
                    nc.sync.dma_start(
                        out=rcb[64:128, :],
                        in_=rscr2[1:2, :].to_broadcast((64, 2048)),
                    )
                    blk[qb] = {"stg": stg, "rcb": rcb}

                def emit_norm(qb, sts):
                    b = blk[qb]
                    if "ctxT" not in b:
                        b["ctxT"] = ctp.tile([128, 2048], BF16, tag="ctxT",
                                             name=f"ctxT{qb}")
                    def chunk(t, st):
                        # cols {pair*512 + st*128 .. +128, pair=0..4}
                        return t.rearrange("p (a b) -> p a b", a=4)[
                            :, :, st * 128: st * 128 + 128]
                    for st in sts:
                        nc.vector.tensor_mul(
                            out=chunk(b["ctxT"][:, :], st),
                            in0=chunk(b["stg"][:, :], st),
                            in1=chunk(b["rcb"][:, :], st),
                        )

                def emit_outproj(qb, sts, final=False):
                    b = blk[qb]
                    ctxT = b["ctxT"]
                    if "po" not in b:
                        b["po"] = osb.tile([128, 1024], F32, tag="ot",
                                           name=f"ot{qb}")
                    po = b["po"]
                    if final:
                        blk.pop(qb)
                    for st in sts:
                        for pair in range(4):
                            nc.tensor.matmul(
                                out=ctx_ps[:, st * 512: st * 512 + 256],
                                lhsT=ctxT[:, pair * 512 + st * 128:
                                          pair * 512 + st * 128 + 128],
                                rhs=wo_sb[:, pair * 256: pair * 256 + 256],
                                start=(pair == 0), stop=(pair == 3),
                                tile_position=(0, 0),
                                skip_group_check=True,
                            )
                    for st in sts:
                        nc.scalar.copy(
                            out=po[:, st * 256: st * 256 + 256],
                            in_=ctx_ps[:, st * 512: st * 512 + 256],
                        )
                        nc.sync.dma_start(
                            out=out[qb * QBS + st * 128: qb * QBS + st * 128 + 128, :],
                            in_=po[:, st * 256: st * 256 + 256],
                        )

                # ctx schedule: each block's first ctx batches are delayed
                # by D extra slots (scores keep the PE busy meanwhile) so the
                # PREVIOUS block's out-proj can finish with the ctx banks
                # free; the backlog catches up at 2 ctx batches per slot.
                D = 7

                def ctx_slot(ci):
                    qb, l = divmod(ci, 32)
                    if l < 2 * D:
                        return qb * 32 + CTX_DELAY + D + (l + 1) // 2
                    return ci + CTX_DELAY

                ctx_by_slot = {}
                for ci in range(ng):
                    ctx_by_slot.setdefault(ctx_slot(ci), []).append(ci)
                last_emit = {qb: ctx_slot(qb * 32 + 31) for qb in range(NQB)}
                n_slots = (max(last_emit.values()) + PROJ_OFF + 1) if ng else 0
                if MAX_GI is not None:
                    n_slots = min(n_slots, MAX_GI)
                for gi in range(n_slots):
                    if gi < ng:
                        emit_scores_exp(gi)
                    for ci in ctx_by_slot.get(gi, []):
                        emit_ctx(ci)
                    # staggered boundary work per finished block
                    for qb in range(NQB):
                        base = last_emit[qb]
                        if gi == base + EV_OFF:
                            emit_evict_recip(qb)
                        elif gi == base + NORM_OFF:
                            emit_norm(qb, [0, 1])
                        elif gi == base + NORM_OFF + 1:
                            emit_norm(qb, [2, 3])
                            emit_outproj(qb, [0])
                        elif gi == base + PROJ_OFF:
                            emit_outproj(qb, [1, 2, 3], final=True)
                scp_cm.__exit__(None, None, None)
    if legalize:
        _legalize_sync_waits(nc)
    return nc


_NC_CACHE = None


def _get_nc():
    global _NC_CACHE
    if _NC_CACHE is None:
        _NC_CACHE = _build_nc()
    return _NC_CACHE


def _ks_layout(a, nk, cols):
    """[nk*128, cols] -> [128, nk*cols] with [p, k*cols+c] = a[k*128+p, c]."""
    return np.ascontiguousarray(
        a.reshape(nk, 128, cols).transpose(1, 0, 2).reshape(128, nk * cols)
    )


def _prep_in_maps(x, w_qkv, b_qkv, w_out, b_out):
    x = np.asarray(x, dtype=np.float32)
    w_qkv = np.asarray(w_qkv, dtype=np.float32)
    b_qkv = np.asarray(b_qkv, dtype=np.float32)
    w_out = np.asarray(w_out, dtype=np.float32)
    b_out = np.asarray(b_out, dtype=np.float32)

    wqk_l = _ks_layout(w_qkv[:, : 2 * H], 2, 512).astype(NPBF16)

    # q/k bias as per-partition vectors, one column per phase-1 tile
    bqk_t = np.ascontiguousarray(
        b_qkv[: 2 * H].reshape(4, 128).T).astype(np.float32)

    # v weights: head-pair slots of 66: [v_2p |1| v_2p+1 |1] (ones via bias row)
    wpad = np.zeros((H, 264), np.float32)
    bvr = np.zeros((1, 264), np.float32)
    for h in range(NH):
        c0 = (h // 2) * 66 + (h % 2) * 33
        wpad[:, c0: c0 + 32] = w_qkv[:, 2 * H + h * HD: 2 * H + (h + 1) * HD]
        bvr[0, c0: c0 + 32] = b_qkv[2 * H + h * HD: 2 * H + (h + 1) * HD]
        bvr[0, c0 + 32] = 1.0
    wv_l = _ks_layout(wpad, 2, 264).astype(NPBF16)

    # w_out rows permuted into ctxT slot layout: per pair-block of 128 rows:
    # [head 2p (32) | b_out row (pair 0 only) | 31 zeros | head 2p+1 (32) | 32 zeros]
    wo_perm = np.zeros((512, H), np.float32)
    for pair in range(4):
        r0 = pair * 128
        wo_perm[r0: r0 + 32, :] = w_out[(2 * pair) * HD: (2 * pair + 1) * HD, :]
        wo_perm[r0 + 64: r0 + 96, :] = w_out[(2 * pair + 1) * HD: (2 * pair + 2) * HD, :]
    wo_perm[32, :] = b_out  # ctxT row 32 of pair 0 is rs*1/rs = 1
    wo_l = _ks_layout(wo_perm, 4, 256).astype(NPBF16)

    shared = {
        "wqk": wqk_l,
        "wv": wv_l,
        "bqk": bqk_t,
        "bv": bvr.astype(NPBF16),
        "wo": wo_l,
        "ones": np.ones((1, 512), NPBF16),
    }
    in_maps = []
    for b in range(B):
        xtm = _ks_layout(np.ascontiguousarray(x[b].T), 2, S).astype(NPBF16)
        in_maps.append({"xt": xtm, **shared})
    return in_maps


def kernel(x, w_qkv, b_qkv, w_out, b_out):
    in_maps = _prep_in_maps(x, w_qkv, b_qkv, w_out, b_out)
    nc = _get_nc()
    res = run_bass_kernel_spmd(nc, in_maps, list(range(N_CORES)), **TRACE_OPTS)
    global LAST_RESULT
    LAST_RESULT = res
    return np.stack([res.results[b]["out"] for b in range(B)], axis=0)
